# revision 2
# baseline (speedup 1.0000x reference)
"""Trainium2 Bass kernel for nn_Block_68719476955 — all-fp8-DoubleRow version.

Math: with H=1 the attention softmax is over a singleton axis, so the whole
attention reduces to x @ w_kv + b_kv.

All three matmuls run as fp8-e4m3 DoubleRow pair-passes with digit
compensation:
  kv, fc: every k-chunk pair (c,c') gets 3 passes computing
      (Ah+Al)@Wh + Ah@Wl ~= A@W   (~bf16 accuracy, 0.75x bf16 pass count)
  mproj: first J2=54 k-chunks activation-compensated (2 passes per pair,
    rows are hi/lo digits of SA*(u-MU), cells W8 reused); last 10 chunks
    plain. Host folds MU*colsum(dec(W8)) (shift) and MU*colsum(Werr)
    (mu-trick, data-free) into b_mproj.

Distribution: data-parallel, 1024 tokens/core, token tiles (512, 384, 128)
lockstep per weight block for kv+LN1 and fc; mproj in two stages
(t0 alone, then t1+t2) so each stage's LN2+normalize+store hides under the
next stage's matmuls. Residual x streamed as bf16; weights partition-major
in DRAM (4KB contiguous per-partition rows) for full-rate DMA.

Emulated end-to-end rel err (emul4.py, J2=56): 1.938e-2 vs the 2e-2 gate.
"""

import numpy as np
import ml_dtypes
from contextlib import ExitStack

import concourse.bacc as bacc
import concourse.mybir as mybir
import concourse.tile as tile
from concourse.bass_utils import run_bass_kernel_spmd

P = 128
B, S, E = 4, 2048, 2048
H4 = 4 * E
NCORES = 8
TOK = (B * S) // NCORES    # 1024 tokens per core
TS = (512, 384, 128)       # token tiles
TOFF = (0, 512, 896)
NT = 3
EO = E // P                # 16
FO = H4 // P               # 64
LN_EPS = 1e-5
SA = 8.0                   # activation digit scale
SW = 64.0                  # weight digit scale
PS = SA * SW               # product scale of every DR slot
MU = 0.2423                # u-shift / mu-trick constant (design param)
J2 = 56                    # mproj k-chunks with activation compensation
JP = FO - J2               # plain mproj k-chunks (10)
NUR = 2 * J2 + JP          # u digit rows (118)
NWMP = J2 // 2 + JP // 2   # mproj weight pairs per block (32)

F32 = mybir.dt.float32
BF16 = mybir.dt.bfloat16
F8 = mybir.dt.float8e4
DRM = mybir.MatmulPerfMode.DoubleRow
AF = mybir.ActivationFunctionType
ALU = mybir.AluOpType
E4NP = ml_dtypes.float8_e4m3

_CACHED_NC = {}


def _build(ln2_trivial):
    LN2_TRIVIAL = ln2_trivial
    nc = bacc.Bacc(None, target_bir_lowering=False)

    xb_d = nc.dram_tensor("xb", [E, TOK], BF16, kind="ExternalInput")
    xd_d = nc.dram_tensor("xd", [P, 32, TOK], F8, kind="ExternalInput")
    wkv_d = nc.dram_tensor("wkv", [P, EO, EO, 2, P], F8, kind="ExternalInput")
    wfc_d = nc.dram_tensor("wfc", [P, FO, EO, 2, P], F8, kind="ExternalInput")
    wmp_d = nc.dram_tensor("wmp", [P, EO, NWMP, 2, P], F8, kind="ExternalInput")
    bkv_d = nc.dram_tensor("bkv", [P, EO], F32, kind="ExternalInput")
    bfc_d = nc.dram_tensor("bfc", [P, FO], F32, kind="ExternalInput")
    bmp_d = nc.dram_tensor("bmp", [P, EO], F32, kind="ExternalInput")
    g2_d = nc.dram_tensor("g2", [P, EO], F32, kind="ExternalInput")
    b2_d = nc.dram_tensor("b2", [P, EO], F32, kind="ExternalInput")
    out_d = nc.dram_tensor("out", [E, TOK], BF16, kind="ExternalOutput")

    with tile.TileContext(nc) as tc, ExitStack() as ctx:
        consts = ctx.enter_context(tc.tile_pool(name="consts", bufs=1))
        # 32-row fp8 arenas per tile: x digits -> h digits -> (as bf16) v+x
        d32p = ctx.enter_context(tc.tile_pool(name="d32p", bufs=1))
        # big arena per tile: r1 (f32, 16 rows) -> u digits (fp8, 118 rows)
        bigp = ctx.enter_context(tc.tile_pool(name="bigp", bufs=1))
        wp = ctx.enter_context(tc.tile_pool(name="wp", bufs=5))
        f32t = ctx.enter_context(tc.tile_pool(name="f32t", bufs=3))
        xbp = ctx.enter_context(tc.tile_pool(name="xbp", bufs=3))
        sqp = ctx.enter_context(tc.tile_pool(name="sqp", bufs=4))
        accp = ctx.enter_context(tc.tile_pool(name="accp", bufs=1))
        stp = ctx.enter_context(tc.tile_pool(name="stp", bufs=1))
        bcp = ctx.enter_context(tc.tile_pool(name="bcp", bufs=2))
        psmm = ctx.enter_context(tc.tile_pool(name="psmm", bufs=4, space="PSUM"))
        psst = ctx.enter_context(tc.tile_pool(name="psst", bufs=2, space="PSUM"))

        def tsl(t):
            return slice(TOFF[t], TOFF[t] + TS[t])

        # ---- constants (gpsimd queue) ----
        bkv_t = consts.tile([P, EO], F32)
        nc.gpsimd.dma_start(bkv_t[:], bkv_d[:, :])
        bfc_t = consts.tile([P, FO], F32)
        nc.gpsimd.dma_start(bfc_t[:], bfc_d[:, :])
        bmp_t = consts.tile([P, EO], F32)
        nc.gpsimd.dma_start(bmp_t[:], bmp_d[:, :])
        g2_t = consts.tile([P, EO], F32)
        nc.gpsimd.dma_start(g2_t[:], g2_d[:, :])
        b2_t = consts.tile([P, EO], F32)
        nc.gpsimd.dma_start(b2_t[:], b2_d[:, :])
        ones_col = consts.tile([P, 1], BF16)
        nc.vector.memset(ones_col[:], 1.0 / E)
        eps_t = consts.tile([1, 1], F32)
        nc.vector.memset(eps_t[:], LN_EPS / (SA * SA))
        eps2_t = consts.tile([1, 1], F32)
        nc.vector.memset(eps2_t[:], LN_EPS)
        negmu_t = consts.tile([P, 1], F32)
        nc.vector.memset(negmu_t[:], -MU * SA)
        sa_t = consts.tile([P, 1], F32)
        nc.vector.memset(sa_t[:], SA)

        # ---- x digit tiles: groups of 4 rows (Ah,Ah',Al,Al') per pair ----
        xds = []
        for t in range(NT):
            xd = d32p.tile([P, 32, TS[t]], F8, tag=f"d32_{t}", name=f"xd{t}")
            xds.append(xd)
        dmaq = (nc.scalar, nc.gpsimd)

        def load_xd(t):
            for g in range(8):
                dmaq[g % 2].dma_start(xds[t][:, 4 * g:4 * g + 4, :],
                                      xd_d[:, 4 * g:4 * g + 4, tsl(t)])

        load_xd(0)

        # warm MMs: keep PE busy while the first DMAs land (p-state ramp)
        warm_rhs = consts.tile([P, 512], BF16)
        nc.vector.memset(warm_rhs[:], 1.0)
        warm_ps = psst.tile([1, 512], F32, tag="pss")
        for _ in range(20):
            nc.tensor.matmul(warm_ps[:], lhsT=ones_col[:],
                             rhs=warm_rhs[:], start=True, stop=True)

        def hilo_group(ps, wt, rows, T):
            """24 DR passes: (Ah+Al)@Wh + Ah@Wl for 8 chunk pairs."""
            for g in range(8):
                hi = rows[:, 4 * g:4 * g + 2, :]
                lo = rows[:, 4 * g + 2:4 * g + 4, :]
                nc.tensor.matmul(ps[:, :T], lhsT=wt[:, 2 * g, :, :], rhs=hi,
                                 start=(g == 0), stop=False, perf_mode=DRM)
                nc.tensor.matmul(ps[:, :T], lhsT=wt[:, 2 * g, :, :], rhs=lo,
                                 start=False, stop=False, perf_mode=DRM)
                nc.tensor.matmul(ps[:, :T], lhsT=wt[:, 2 * g + 1, :, :], rhs=hi,
                                 start=False, stop=(g == 7), perf_mode=DRM)

        def ln_finalize(t, ps_sum, ps_sq, bc_out=None, coff=0,
                        for_ln2=False):
            """bc [P,4,T] bf16: rows (m*rstd*SA, m*rstd*SA, rstd*SA, rstd*SA)
            so chunk-PAIR ops can slice bc[:,0:2] / bc[:,2:4] directly.
            ps_sum/ps_sq arrive pre-divided by E (ones = 1/E)."""
            T = TS[t]
            st = stp.tile([1, 4, 512], BF16, tag="st")
            nc.vector.tensor_scalar_mul(st[:, 1, :T], ps_sum[:, :T], 1.0)
            nc.vector.tensor_mul(out=st[:, 3, :T], in0=st[:, 1, :T],
                                 in1=st[:, 1, :T])
            nc.vector.tensor_tensor(st[:, 2, :T], ps_sq[:, :T], st[:, 3, :T],
                                    ALU.subtract)
            if for_ln2:
                nc.scalar.activation(st[:, 3, :T], st[:, 2, :T], AF.Sqrt,
                                     bias=eps2_t[:], scale=1.0)
            else:
                nc.scalar.activation(st[:, 3, :T], st[:, 2, :T], AF.Sqrt,
                                     bias=eps_t[:], scale=1.0 / (SA * SA))
            with nc.allow_low_precision(reason="bf16 rstd: ~0.2% scale "
                                        "error, well inside the fp8 budget"):
                nc.vector.reciprocal(out=st[:, 2, :T], in_=st[:, 3, :T])
            nc.vector.tensor_scalar_mul(st[:, 3, :T], st[:, 2, :T], 1.0)
            nc.vector.tensor_mul(out=st[:, 0, :T], in0=st[:, 1, :T],
                                 in1=st[:, 2, :T])
            nc.vector.tensor_scalar_mul(st[:, 1, :T], st[:, 0, :T], 1.0)
            bc = bc_out
            if bc is None:
                bc = bcp.tile([P, 4, 512], BF16, tag="bc")
            # rstd rows first: the chunk-pair muls only need bc[:,2:4]
            nc.gpsimd.partition_broadcast(bc[:, 2:4, coff:coff + T],
                                          st[:, 2:4, :T])
            nc.gpsimd.partition_broadcast(bc[:, 0:2, coff:coff + T],
                                          st[:, 0:2, :T])
            return bc

        def stats_chain(run, t, val_f32, m):
            """bf16 running sums of values (DVE) and squares (ACT + DVE)."""
            T = TS[t]
            if m == 0:
                pr = accp.tile([P, TS[t]], BF16, tag=f"pr{t}")
                nc.vector.tensor_scalar_mul(pr[:, :T], val_f32, 1.0)
                pq = accp.tile([P, TS[t]], BF16, tag=f"pq{t}")
                nc.scalar.activation(pq[:, :T], val_f32, AF.Square)
                run["pr"], run["pq"] = pr, pq
            else:
                sq = sqp.tile([P, 512], BF16, tag="sq", bufs=3)
                nc.scalar.activation(sq[:, :T], val_f32, AF.Square)
                nc.vector.tensor_tensor(run["pr"][:, :T], run["pr"][:, :T],
                                        val_f32, ALU.add)
                nc.vector.tensor_tensor(run["pq"][:, :T], run["pq"][:, :T],
                                        sq[:, :T], ALU.add)

        def stats_mms(run, t):
            T = TS[t]
            ps_sum = psst.tile([1, 512], F32, tag="pss")
            ps_sq = psst.tile([1, 512], F32, tag="psq")
            nc.tensor.matmul(ps_sum[:, :T], lhsT=ones_col[:],
                             rhs=run["pr"][:, :T], start=True, stop=True)
            nc.tensor.matmul(ps_sq[:, :T], lhsT=ones_col[:],
                             rhs=run["pq"][:, :T], start=True, stop=True)
            return ps_sum, ps_sq

        # ---------------- LN1 finalize + h digits (per tile) -------------
        # h digit rows mirror the x layout: chunk c -> hi row 4*(c//2)+(c%2),
        # lo row = hi row + 2. LN1's gamma/beta are folded into wfc/bfc on the
        # host, so digits quantize the bare normalized SA*h (bf16 chain, 2x).
        hds = [None] * NT
        pending = []   # deferred digit-pair closures, drained 1-3 per block

        def drain(n):
            for _ in range(min(n, len(pending))):
                pending.pop(0)()

        def emit_ln1_digits(t):
            sA, qA = stats_mms(runA[t], t)
            bc = ln_finalize(t, sA, qA)
            T = TS[t]
            hd = d32p.tile([P, 32, T], F8, tag=f"d32_{t}", name=f"hd{t}")
            hds[t] = hd

            def digit_pair(g, t=t, bc=bc, hd=hd, T=T):
                tm = sqp.tile([P, 2, 512], BF16, tag="dig", bufs=3)
                nc.vector.tensor_mul(out=tm[:, :, :T],
                                     in0=r1s[t][:, 2 * g:2 * g + 2, :],
                                     in1=bc[:, 2:4, :T])
                t2 = sqp.tile([P, 2, 512], BF16, tag="dig", bufs=3)
                nc.vector.tensor_tensor(t2[:, :, :T], tm[:, :, :T],
                                        bc[:, 0:2, :T], ALU.subtract)
                nc.scalar.activation(hd[:, 4 * g:4 * g + 2, :], t2[:, :, :T],
                                     AF.Identity)
                nc.vector.tensor_tensor(hd[:, 4 * g + 2:4 * g + 4, :],
                                        t2[:, :, :T],
                                        hd[:, 4 * g:4 * g + 2, :],
                                        ALU.subtract)
            for g in range(EO // 2):
                pending.append(lambda g=g: digit_pair(g))

        # ---------------- phase A: kv matmul + residual ----------------
        # tile offsets (0,1,2): tile t processes block mb - t, so t0 finishes
        # early and its LN1/digit chain overlaps the other tiles' tail blocks.
        r1s, runA = [], [dict() for _ in range(NT)]
        for t in range(NT):
            r1 = bigp.tile([P, EO, TS[t]], BF16, tag=f"big_{t}", name=f"r1{t}")
            r1s.append(r1)
        def a_block(t, m, wt):
            T = TS[t]
            ps = psmm.tile([P, 512], F32, tag="ps")
            hilo_group(ps, wt, xds[t], T)
            xc = xbp.tile([P, 512], BF16, tag="xb")
            (nc.sync if t == 0 else nc.gpsimd).dma_start(
                xc[:, :T], xb_d[m * P:(m + 1) * P, tsl(t)])
            t1 = f32t.tile([P, 512], F32, tag="f32")
            nc.scalar.activation(t1[:, :T], ps[:, :T], AF.Identity,
                                 bias=bkv_t[:, m:m + 1], scale=1.0 / PS)
            nc.vector.tensor_add(out=r1s[t][:, m, :], in0=t1[:, :T],
                                 in1=xc[:, :T])
            stats_chain(runA[t], t, r1s[t][:, m, :], m)

        # pass 1: tile 0 alone; its LN1 + digit chain then overlaps pass 2.
        for m in range(EO):
            wt = wp.tile([P, EO, 2, P], F8, tag="w16")
            if m == 0:
                for q in range(4):
                    nc.sync.dma_start(wt[:, 4 * q:4 * q + 4, :, :],
                                      wkv_d[:, m, 4 * q:4 * q + 4, :, :])
            else:
                nc.sync.dma_start(wt[:], wkv_d[:, m])
            a_block(0, m, wt)
            if m == 6:
                load_xd(1)
            if m == 10:
                load_xd(2)
        emit_ln1_digits(0)
        # pass 2: tiles 1,2 lockstep (t2 lagging one block); kv weights are
        # cheap enough to stream a second time.
        wts_a = {}
        for mb in range(EO + 1):
            if mb < EO:
                wt = wp.tile([P, EO, 2, P], F8, tag="w16")
                wts_a[mb] = wt
                nc.sync.dma_start(wt[:], wkv_d[:, mb])
            for t, off in ((1, 0), (2, 1)):
                m = mb - off
                if not (0 <= m < EO):
                    continue
                a_block(t, m, wts_a[m])
                if m == EO - 1:
                    emit_ln1_digits(t)
            drain(1)


        # ---------------- phase B1: fc matmul + gelu + u digits ----------
        uds = []
        for t in range(NT):
            ud = bigp.tile([P, NUR, TS[t]], F8, tag=f"big_{t}", name=f"ud{t}")
            uds.append(ud)
        FSKIP = (0, 4, 6)

        def fc_block(t, ma, wt):
                T = TS[t]
                ps = psmm.tile([P, 512], F32, tag="ps")
                hilo_group(ps, wt, hds[t], T)
                if ma < J2:
                    uf = f32t.tile([P, 512], F32, tag="f32")
                    nc.scalar.activation(uf[:, :T], ps[:, :T], AF.Gelu,
                                         bias=bfc_t[:, ma:ma + 1],
                                         scale=1.0 / PS)
                    nc.scalar.activation(uds[t][:, ma, :], uf[:, :T],
                                         AF.Identity, bias=negmu_t[:],
                                         scale=SA)
                    t3 = f32t.tile([P, 512], F32, tag="f32")
                    nc.vector.tensor_scalar(t3[:, :T], uf[:, :T], sa_t[:],
                                            negmu_t[:], ALU.mult, ALU.add)
                    nc.vector.tensor_tensor(uds[t][:, J2 + ma, :], t3[:, :T],
                                            uds[t][:, ma, :], ALU.subtract)
                else:
                    nc.scalar.activation(uds[t][:, 2 * J2 + (ma - J2), :],
                                         ps[:, :T], AF.Gelu,
                                         bias=bfc_t[:, ma:ma + 1],
                                         scale=1.0 / PS)

        for mb in range(FO):
            wt = wp.tile([P, EO, 2, P], F8, tag="w16")
            nc.sync.dma_start(wt[:], wfc_d[:, mb])
            for t in range(NT):
                if mb >= FSKIP[t]:
                    fc_block(t, mb, wt)
            drain(3)
        # catch-up: the first blocks t1/t2 skipped, with re-streamed weights
        for cb in range(max(FSKIP)):
            wt = wp.tile([P, EO, 2, P], F8, tag="w16")
            nc.sync.dma_start(wt[:], wfc_d[:, cb])
            for t in (1, 2):
                if cb < FSKIP[t]:
                    fc_block(t, cb, wt)

        # -------- phase B2 + C: mproj + LN2 + output, two stages --------
        # stage 2 (t1,t2) shares one 512-column v2f tile and one bc tile so
        # the exposed tail normalizes and stores both tiles full-width.
        v2fs = [None] * NT
        OFFV = (0, 0, TS[1])
        bc2s = [None] * NT
        outq = (nc.gpsimd, nc.sync)

        def phase_c_begin(t, runB):
            sB, qB = stats_mms(runB, t)
            bc2s[t] = ln_finalize(t, sB, qB, for_ln2=True)

        def phase_c_pair(t, g, tailq=False, wide=False):
            """normalize chunks 2g, 2g+1 of tile t in-place into the (dead)
            v2f rows; with wide=True the op covers the merged t1|t2 columns
            and stores go one-per-chunk across both tiles."""
            T = 512 if wide else TS[t]
            co = 0 if wide else OFFV[t]
            bc = bc2s[t]
            q = (nc.sync, nc.scalar, nc.gpsimd) if tailq else outq
            vsl = v2fs[t][:, 2 * g:2 * g + 2, co:co + T]
            tm = sqp.tile([P, 2, 512], BF16, tag="dig", bufs=3)
            nc.vector.tensor_mul(out=tm[:, :, :T], in0=vsl,
                                 in1=bc[:, 2:4, co:co + T])
            if LN2_TRIVIAL:
                nc.vector.tensor_tensor(vsl, tm[:, :, :T],
                                        bc[:, 0:2, co:co + T], ALU.subtract)
            else:
                for i in (0, 1):
                    m = 2 * g + i
                    t2 = sqp.tile([P, 512], BF16, tag="dg1")
                    nc.vector.tensor_tensor(t2[:, :T], tm[:, i, :T],
                                            bc[:, i, co:co + T], ALU.subtract)
                    nc.scalar.activation(v2fs[t][:, m, co:co + T], t2[:, :T],
                                         AF.Identity, bias=b2_t[:, m:m + 1],
                                         scale=g2_t[:, m:m + 1])
            cs = slice(TOFF[1], TOK) if wide else tsl(t)
            for i in (0, 1):
                m = 2 * g + i
                q[m % len(q)].dma_start(out_d[m * P:(m + 1) * P, cs],
                                        v2fs[t][:, m, co:co + T])

        def mp_pass_group(ps, wta, wtb, t, T):
            def cell(j):
                return wta[:, j, :, :] if j < EO else wtb[:, j - EO, :, :]

            for j in range(J2 // 2):
                nc.tensor.matmul(ps[:, :T], lhsT=cell(j),
                                 rhs=uds[t][:, 2 * j:2 * j + 2, :],
                                 start=(j == 0), stop=False, perf_mode=DRM)
                nc.tensor.matmul(ps[:, :T], lhsT=cell(j),
                                 rhs=uds[t][:, J2 + 2 * j:J2 + 2 * j + 2, :],
                                 start=False, stop=False, perf_mode=DRM)
            for p in range(JP // 2):
                nc.tensor.matmul(ps[:, :T], lhsT=cell(J2 // 2 + p),
                                 rhs=uds[t][:, 2 * J2 + 2 * p:
                                            2 * J2 + 2 * p + 2, :],
                                 start=False, stop=(p == JP // 2 - 1),
                                 perf_mode=DRM)

        prev_tiles = []
        prev_runB = {}
        for stage in ((0,), (1, 2)):
            runB = {t: dict() for t in stage}
            if stage == (0,):
                v2fs[0] = d32p.tile([P, EO, TS[0]], BF16, tag="d32_0",
                                    name="v2f0")
            else:
                v2f12 = bigp.tile([P, EO, 512], BF16, tag="big_0",
                                  name="v2f12")
                v2fs[1] = v2f12
                v2fs[2] = v2f12
            for mo in range(EO):
                wta = wp.tile([P, EO, 2, P], F8, tag="w16", name="wta")
                nc.sync.dma_start(wta[:], wmp_d[:, mo, :EO])
                wtb = wp.tile([P, EO, 2, P], F8, tag="w16", name="wtb")
                nc.sync.dma_start(wtb[:], wmp_d[:, mo, EO:])
                for t in stage:
                    T = TS[t]
                    co = OFFV[t]
                    ps = psmm.tile([P, 512], F32, tag="ps")
                    mp_pass_group(ps, wta, wtb, t, T)
                    xc = xbp.tile([P, 512], BF16, tag="xb")
                    nc.gpsimd.dma_start(xc[:, :T],
                                        xb_d[mo * P:(mo + 1) * P, tsl(t)])
                    tv = f32t.tile([P, 512], F32, tag="f32")
                    nc.scalar.activation(tv[:, :T], ps[:, :T], AF.Identity,
                                         bias=bmp_t[:, mo:mo + 1],
                                         scale=1.0 / PS)
                    vrow = v2fs[t][:, mo, co:co + T]
                    nc.vector.tensor_add(out=vrow, in0=tv[:, :T],
                                         in1=xc[:, :T])
                    stats_chain(runB[t], t, vrow, mo)
                for tp in prev_tiles:
                    if mo == 0:
                        phase_c_begin(tp, prev_runB[tp])
                    if mo % 2 == 0:
                        phase_c_pair(tp, mo // 2)

            prev_tiles = list(stage)
            prev_runB = runB
        # tail: LN2 + normalize + store for the merged t1|t2 tile, full-width
        bc12 = bcp.tile([P, 4, 512], BF16, tag="bc")
        for tp in prev_tiles:
            sB, qB = stats_mms(prev_runB[tp], tp)
            ln_finalize(tp, sB, qB, bc_out=bc12, coff=OFFV[tp],
                        for_ln2=True)
            bc2s[tp] = bc12
        for g in range(EO // 2):
            phase_c_pair(1, g, tailq=True, wide=True)

    nc.compile()
    return nc


def _get_nc(ln2_trivial=True):
    if ln2_trivial not in _CACHED_NC:
        _CACHED_NC[ln2_trivial] = _build(ln2_trivial)
    return _CACHED_NC[ln2_trivial]


def _q(x):
    return x.astype(E4NP)


def _dec(x):
    return x.astype(np.float32)


def _prep_inputs(x, w_kv, b_kv, w_fc, b_fc, w_mproj, b_mproj,
                 ln1_g, ln1_b, ln2_g, ln2_b):
    f32 = np.float32
    x_flat = np.ascontiguousarray(np.asarray(x, f32).reshape(B * S, E))

    def pack_hilo(W, KO, NO):
        """-> [P, NO, KO, 2, P] fp8; pair 2j=(Wh_2j,Wh_2j+1), 2j+1=lo pair."""
        Ws = (np.asarray(W, f32) * SW).reshape(KO, P, NO, P)
        Wh = _q(Ws)
        Wl = _q(Ws - _dec(Wh))
        Whp = Wh.reshape(KO // 2, 2, P, NO, P).transpose(3, 2, 0, 1, 4)
        Wlp = Wl.reshape(KO // 2, 2, P, NO, P).transpose(3, 2, 0, 1, 4)
        arr = np.stack([Whp, Wlp], axis=3)          # [NO,P,KO/2,2,2,P]
        arr = arr.reshape(NO, P, KO, 2, P).transpose(1, 0, 2, 3, 4)
        return np.ascontiguousarray(arr)            # [P,NO,KO,2,P]

    wkv_t = pack_hilo(w_kv, EO, EO)
    # fold LN1 affine into fc: z = (g1*hhat + b1) @ wfc + bfc
    #   = hhat @ (g1[:,None]*wfc) + (bfc + b1 @ wfc); digits quantize SA*hhat.
    g1 = np.asarray(ln1_g, f32)
    b1 = np.asarray(ln1_b, f32)
    wfc_f = np.asarray(w_fc, f32) * g1[:, None]
    bfc_f = np.asarray(b_fc, f32) + b1 @ np.asarray(w_fc, f32)
    wfc_t = pack_hilo(wfc_f, EO, FO)

    Wmp = np.asarray(w_mproj, f32).reshape(FO, P, EO, P)
    wc = Wmp[:J2] * SW
    w8c = _q(wc)
    wpl = Wmp[J2:] * (SA * SW)
    w8p = _q(wpl)
    pairs_c = _dec(w8c).reshape(J2 // 2, 2, P, EO, P).transpose(3, 2, 0, 1, 4)
    pairs_p = _dec(w8p).reshape(JP // 2, 2, P, EO, P).transpose(3, 2, 0, 1, 4)
    wmp_t = np.concatenate([_q(pairs_c), _q(pairs_p)], axis=2)
    wmp_t = np.ascontiguousarray(wmp_t.transpose(1, 0, 2, 3, 4))

    # host bias corrections (data-free)
    err_c = (wc - _dec(w8c)).sum((0, 1)) / SW            # [EO, P]
    err_p = (wpl - _dec(w8p)).sum((0, 1)) / (SA * SW)
    dec_c = _dec(w8c).sum((0, 1)) / SW
    bcorr = MU * (err_c + err_p) + MU * dec_c            # [EO, P]
    bmp_c = np.asarray(b_mproj, f32).reshape(EO, P) + bcorr

    def p2d(v):
        v = np.asarray(v, f32)
        return np.ascontiguousarray(v.reshape(-1, P).T)

    shared = {
        "wkv": wkv_t, "wfc": wfc_t, "wmp": wmp_t,
        "bkv": p2d(b_kv), "bfc": p2d(bfc_f),
        "bmp": np.ascontiguousarray(bmp_c.T),
        "g2": p2d(ln2_g), "b2": p2d(ln2_b),
    }
    in_maps = []
    for c in range(NCORES):
        xT = np.ascontiguousarray(x_flat[c * TOK:(c + 1) * TOK].T)  # [E, TOK]
        xs = xT * np.float32(SA)
        xh = _q(xs)
        xl = _q(xs - _dec(xh))
        xh = xh.reshape(EO, P, TOK)
        xl = xl.reshape(EO, P, TOK)
        # group-of-4 rows: [Ah_2g, Ah_2g+1, Al_2g, Al_2g+1]
        xd = np.empty((32, P, TOK), E4NP)
        for g in range(8):
            xd[4 * g] = xh[2 * g]
            xd[4 * g + 1] = xh[2 * g + 1]
            xd[4 * g + 2] = xl[2 * g]
            xd[4 * g + 3] = xl[2 * g + 1]
        in_maps.append({
            "xb": xT.astype(ml_dtypes.bfloat16),
            "xd": np.ascontiguousarray(xd.transpose(1, 0, 2)),
            **shared})
    return in_maps


def _run(inputs, trace=False):
    ln2_trivial = bool(np.all(np.asarray(inputs["ln2_g"]) == 1.0)
                       and np.all(np.asarray(inputs["ln2_b"]) == 0.0))
    nc = _get_nc(ln2_trivial)
    in_maps = _prep_inputs(
        inputs["x"], inputs["w_kv"], inputs["b_kv"], inputs["w_fc"],
        inputs["b_fc"], inputs["w_mproj"], inputs["b_mproj"],
        inputs["ln1_g"], inputs["ln1_b"], inputs["ln2_g"], inputs["ln2_b"])
    res = run_bass_kernel_spmd(nc, in_maps, core_ids=list(range(NCORES)),
                               trace=trace)
    outs = [np.asarray(res.results[c]["out"]).astype(np.float32).T
            for c in range(NCORES)]
    full = np.concatenate(outs, axis=0).reshape(B, S, E)
    return full, res


def kernel(**inputs) -> np.ndarray:
    out, _ = _run(inputs, trace=False)
    return out


# revision 3
# speedup vs baseline: 1.0046x; 1.0046x over previous
"""Trainium2 Bass kernel for nn_Block_68719476955 — all-fp8-DoubleRow version.

Math: with H=1 the attention softmax is over a singleton axis, so the whole
attention reduces to x @ w_kv + b_kv.

All three matmuls run as fp8-e4m3 DoubleRow pair-passes with digit
compensation:
  kv, fc: every k-chunk pair (c,c') gets 3 passes computing
      (Ah+Al)@Wh + Ah@Wl ~= A@W   (~bf16 accuracy, 0.75x bf16 pass count)
  mproj: first J2=54 k-chunks activation-compensated (2 passes per pair,
    rows are hi/lo digits of SA*(u-MU), cells W8 reused); last 10 chunks
    plain. Host folds MU*colsum(dec(W8)) (shift) and MU*colsum(Werr)
    (mu-trick, data-free) into b_mproj.

Distribution: data-parallel, 1024 tokens/core, token tiles (512, 384, 128)
lockstep per weight block for kv+LN1 and fc; mproj in two stages
(t0 alone, then t1+t2) so each stage's LN2+normalize+store hides under the
next stage's matmuls. Residual x streamed as bf16; weights partition-major
in DRAM (4KB contiguous per-partition rows) for full-rate DMA.

Emulated end-to-end rel err (emul4.py, J2=56): 1.938e-2 vs the 2e-2 gate.
"""

import numpy as np
import ml_dtypes
from contextlib import ExitStack

import concourse.bacc as bacc
import concourse.mybir as mybir
import concourse.tile as tile
from concourse.bass_utils import run_bass_kernel_spmd

P = 128
B, S, E = 4, 2048, 2048
H4 = 4 * E
NCORES = 8
TOK = (B * S) // NCORES    # 1024 tokens per core
TS = (512, 384, 128)       # token tiles
TOFF = (0, 512, 896)
NT = 3
EO = E // P                # 16
FO = H4 // P               # 64
LN_EPS = 1e-5
SA = 8.0                   # activation digit scale
SW = 64.0                  # weight digit scale
PS = SA * SW               # product scale of every DR slot
MU = 0.2423                # u-shift / mu-trick constant (design param)
J2 = 56                    # mproj k-chunks with activation compensation
JP = FO - J2               # plain mproj k-chunks (10)
NUR = 2 * J2 + JP          # u digit rows (118)
NWMP = J2 // 2 + JP // 2   # mproj weight pairs per block (32)

F32 = mybir.dt.float32
BF16 = mybir.dt.bfloat16
F8 = mybir.dt.float8e4
DRM = mybir.MatmulPerfMode.DoubleRow
AF = mybir.ActivationFunctionType
ALU = mybir.AluOpType
E4NP = ml_dtypes.float8_e4m3

_CACHED_NC = {}


def _build(ln2_trivial):
    LN2_TRIVIAL = ln2_trivial
    nc = bacc.Bacc(None, target_bir_lowering=False)

    xb_d = nc.dram_tensor("xb", [E, TOK], BF16, kind="ExternalInput")
    xd_d = nc.dram_tensor("xd", [P, 32, TOK], F8, kind="ExternalInput")
    wkv_d = nc.dram_tensor("wkv", [P, EO, EO, 2, P], F8, kind="ExternalInput")
    wfc_d = nc.dram_tensor("wfc", [P, FO, EO, 2, P], F8, kind="ExternalInput")
    wmp_d = nc.dram_tensor("wmp", [P, EO, NWMP, 2, P], F8, kind="ExternalInput")
    bkv_d = nc.dram_tensor("bkv", [P, EO], F32, kind="ExternalInput")
    bfc_d = nc.dram_tensor("bfc", [P, FO], F32, kind="ExternalInput")
    bmp_d = nc.dram_tensor("bmp", [P, EO], F32, kind="ExternalInput")
    g2_d = nc.dram_tensor("g2", [P, EO], F32, kind="ExternalInput")
    b2_d = nc.dram_tensor("b2", [P, EO], F32, kind="ExternalInput")
    out_d = nc.dram_tensor("out", [E, TOK], BF16, kind="ExternalOutput")

    with tile.TileContext(nc) as tc, ExitStack() as ctx:
        consts = ctx.enter_context(tc.tile_pool(name="consts", bufs=1))
        # 32-row fp8 arenas per tile: x digits -> h digits -> (as bf16) v+x
        d32p = ctx.enter_context(tc.tile_pool(name="d32p", bufs=1))
        # big arena per tile: r1 (f32, 16 rows) -> u digits (fp8, 118 rows)
        bigp = ctx.enter_context(tc.tile_pool(name="bigp", bufs=1))
        wp = ctx.enter_context(tc.tile_pool(name="wp", bufs=5))
        f32t = ctx.enter_context(tc.tile_pool(name="f32t", bufs=3))
        xbp = ctx.enter_context(tc.tile_pool(name="xbp", bufs=3))
        sqp = ctx.enter_context(tc.tile_pool(name="sqp", bufs=4))
        accp = ctx.enter_context(tc.tile_pool(name="accp", bufs=1))
        stp = ctx.enter_context(tc.tile_pool(name="stp", bufs=1))
        bcp = ctx.enter_context(tc.tile_pool(name="bcp", bufs=2))
        psmm = ctx.enter_context(tc.tile_pool(name="psmm", bufs=4, space="PSUM"))
        psst = ctx.enter_context(tc.tile_pool(name="psst", bufs=2, space="PSUM"))

        def tsl(t):
            return slice(TOFF[t], TOFF[t] + TS[t])

        # ---- constants (gpsimd queue) ----
        bkv_t = consts.tile([P, EO], F32)
        nc.gpsimd.dma_start(bkv_t[:], bkv_d[:, :])
        bfc_t = consts.tile([P, FO], F32)
        nc.gpsimd.dma_start(bfc_t[:], bfc_d[:, :])
        bmp_t = consts.tile([P, EO], F32)
        nc.gpsimd.dma_start(bmp_t[:], bmp_d[:, :])
        g2_t = consts.tile([P, EO], F32)
        nc.gpsimd.dma_start(g2_t[:], g2_d[:, :])
        b2_t = consts.tile([P, EO], F32)
        nc.gpsimd.dma_start(b2_t[:], b2_d[:, :])
        ones_col = consts.tile([P, 1], BF16)
        nc.vector.memset(ones_col[:], 1.0 / E)
        eps_t = consts.tile([1, 1], F32)
        nc.vector.memset(eps_t[:], LN_EPS / (SA * SA))
        eps2_t = consts.tile([1, 1], F32)
        nc.vector.memset(eps2_t[:], LN_EPS)
        negmu_t = consts.tile([P, 1], F32)
        nc.vector.memset(negmu_t[:], -MU * SA)
        sa_t = consts.tile([P, 1], F32)
        nc.vector.memset(sa_t[:], SA)

        # ---- x digit tiles: groups of 4 rows (Ah,Ah',Al,Al') per pair ----
        xds = []
        for t in range(NT):
            xd = d32p.tile([P, 32, TS[t]], F8, tag=f"d32_{t}", name=f"xd{t}")
            xds.append(xd)
        dmaq = (nc.scalar, nc.gpsimd)

        def load_xd(t):
            for g in range(8):
                dmaq[g % 2].dma_start(xds[t][:, 4 * g:4 * g + 4, :],
                                      xd_d[:, 4 * g:4 * g + 4, tsl(t)])

        load_xd(0)

        # warm MMs: keep PE busy while the first DMAs land (p-state ramp)
        warm_rhs = consts.tile([P, 512], BF16)
        nc.vector.memset(warm_rhs[:], 1.0)
        warm_ps = psst.tile([1, 512], F32, tag="pss")
        for _ in range(14):
            nc.tensor.matmul(warm_ps[:], lhsT=ones_col[:],
                             rhs=warm_rhs[:], start=True, stop=True)

        def hilo_group(ps, wt, rows, T):
            """24 DR passes: (Ah+Al)@Wh + Ah@Wl for 8 chunk pairs."""
            for g in range(8):
                hi = rows[:, 4 * g:4 * g + 2, :]
                lo = rows[:, 4 * g + 2:4 * g + 4, :]
                nc.tensor.matmul(ps[:, :T], lhsT=wt[:, 2 * g, :, :], rhs=hi,
                                 start=(g == 0), stop=False, perf_mode=DRM)
                nc.tensor.matmul(ps[:, :T], lhsT=wt[:, 2 * g, :, :], rhs=lo,
                                 start=False, stop=False, perf_mode=DRM)
                nc.tensor.matmul(ps[:, :T], lhsT=wt[:, 2 * g + 1, :, :], rhs=hi,
                                 start=False, stop=(g == 7), perf_mode=DRM)

        def ln_finalize(t, ps_sum, ps_sq, bc_out=None, coff=0,
                        for_ln2=False):
            """bc [P,4,T] bf16: rows (m*rstd*SA, m*rstd*SA, rstd*SA, rstd*SA)
            so chunk-PAIR ops can slice bc[:,0:2] / bc[:,2:4] directly.
            ps_sum/ps_sq arrive pre-divided by E (ones = 1/E)."""
            T = TS[t]
            st = stp.tile([1, 4, 512], BF16, tag="st")
            nc.vector.tensor_scalar_mul(st[:, 1, :T], ps_sum[:, :T], 1.0)
            nc.vector.tensor_mul(out=st[:, 3, :T], in0=st[:, 1, :T],
                                 in1=st[:, 1, :T])
            nc.vector.tensor_tensor(st[:, 2, :T], ps_sq[:, :T], st[:, 3, :T],
                                    ALU.subtract)
            if for_ln2:
                nc.scalar.activation(st[:, 3, :T], st[:, 2, :T], AF.Sqrt,
                                     bias=eps2_t[:], scale=1.0)
            else:
                nc.scalar.activation(st[:, 3, :T], st[:, 2, :T], AF.Sqrt,
                                     bias=eps_t[:], scale=1.0 / (SA * SA))
            with nc.allow_low_precision(reason="bf16 rstd: ~0.2% scale "
                                        "error, well inside the fp8 budget"):
                nc.vector.reciprocal(out=st[:, 2, :T], in_=st[:, 3, :T])
            nc.vector.tensor_scalar_mul(st[:, 3, :T], st[:, 2, :T], 1.0)
            nc.vector.tensor_mul(out=st[:, 0, :T], in0=st[:, 1, :T],
                                 in1=st[:, 2, :T])
            nc.vector.tensor_scalar_mul(st[:, 1, :T], st[:, 0, :T], 1.0)
            bc = bc_out
            if bc is None:
                bc = bcp.tile([P, 4, 512], BF16, tag="bc")
            # rstd rows first: the chunk-pair muls only need bc[:,2:4]
            nc.gpsimd.partition_broadcast(bc[:, 2:4, coff:coff + T],
                                          st[:, 2:4, :T])
            nc.gpsimd.partition_broadcast(bc[:, 0:2, coff:coff + T],
                                          st[:, 0:2, :T])
            return bc

        def stats_chain(run, t, val_f32, m):
            """bf16 running sums of values (DVE) and squares (ACT + DVE)."""
            T = TS[t]
            if m == 0:
                pr = accp.tile([P, TS[t]], BF16, tag=f"pr{t}")
                nc.vector.tensor_scalar_mul(pr[:, :T], val_f32, 1.0)
                pq = accp.tile([P, TS[t]], BF16, tag=f"pq{t}")
                nc.scalar.activation(pq[:, :T], val_f32, AF.Square)
                run["pr"], run["pq"] = pr, pq
            else:
                sq = sqp.tile([P, 512], BF16, tag="sq", bufs=3)
                nc.scalar.activation(sq[:, :T], val_f32, AF.Square)
                nc.vector.tensor_tensor(run["pr"][:, :T], run["pr"][:, :T],
                                        val_f32, ALU.add)
                nc.vector.tensor_tensor(run["pq"][:, :T], run["pq"][:, :T],
                                        sq[:, :T], ALU.add)

        def stats_mms(run, t):
            T = TS[t]
            ps_sum = psst.tile([1, 512], F32, tag="pss")
            ps_sq = psst.tile([1, 512], F32, tag="psq")
            nc.tensor.matmul(ps_sum[:, :T], lhsT=ones_col[:],
                             rhs=run["pr"][:, :T], start=True, stop=True)
            nc.tensor.matmul(ps_sq[:, :T], lhsT=ones_col[:],
                             rhs=run["pq"][:, :T], start=True, stop=True)
            return ps_sum, ps_sq

        # ---------------- LN1 finalize + h digits (per tile) -------------
        # h digit rows mirror the x layout: chunk c -> hi row 4*(c//2)+(c%2),
        # lo row = hi row + 2. LN1's gamma/beta are folded into wfc/bfc on the
        # host, so digits quantize the bare normalized SA*h (bf16 chain, 2x).
        hds = [None] * NT
        pending = []   # deferred digit-pair closures, drained 1-3 per block

        def drain(n):
            for _ in range(min(n, len(pending))):
                pending.pop(0)()

        def emit_ln1_digits(t):
            sA, qA = stats_mms(runA[t], t)
            bc = ln_finalize(t, sA, qA)
            T = TS[t]
            hd = d32p.tile([P, 32, T], F8, tag=f"d32_{t}", name=f"hd{t}")
            hds[t] = hd

            def digit_pair(g, t=t, bc=bc, hd=hd, T=T):
                tm = sqp.tile([P, 2, 512], BF16, tag="dig", bufs=3)
                nc.vector.tensor_mul(out=tm[:, :, :T],
                                     in0=r1s[t][:, 2 * g:2 * g + 2, :],
                                     in1=bc[:, 2:4, :T])
                t2 = sqp.tile([P, 2, 512], BF16, tag="dig", bufs=3)
                nc.vector.tensor_tensor(t2[:, :, :T], tm[:, :, :T],
                                        bc[:, 0:2, :T], ALU.subtract)
                nc.scalar.activation(hd[:, 4 * g:4 * g + 2, :], t2[:, :, :T],
                                     AF.Identity)
                nc.vector.tensor_tensor(hd[:, 4 * g + 2:4 * g + 4, :],
                                        t2[:, :, :T],
                                        hd[:, 4 * g:4 * g + 2, :],
                                        ALU.subtract)
            for g in range(EO // 2):
                pending.append(lambda g=g: digit_pair(g))

        # ---------------- phase A: kv matmul + residual ----------------
        # tile offsets (0,1,2): tile t processes block mb - t, so t0 finishes
        # early and its LN1/digit chain overlaps the other tiles' tail blocks.
        r1s, runA = [], [dict() for _ in range(NT)]
        for t in range(NT):
            r1 = bigp.tile([P, EO, TS[t]], BF16, tag=f"big_{t}", name=f"r1{t}")
            r1s.append(r1)
        def a_block(t, m, wt):
            T = TS[t]
            ps = psmm.tile([P, 512], F32, tag="ps")
            hilo_group(ps, wt, xds[t], T)
            xc = xbp.tile([P, 512], BF16, tag="xb")
            (nc.sync if t == 0 else nc.gpsimd).dma_start(
                xc[:, :T], xb_d[m * P:(m + 1) * P, tsl(t)])
            t1 = f32t.tile([P, 512], F32, tag="f32")
            nc.scalar.activation(t1[:, :T], ps[:, :T], AF.Identity,
                                 bias=bkv_t[:, m:m + 1], scale=1.0 / PS)
            nc.vector.tensor_add(out=r1s[t][:, m, :], in0=t1[:, :T],
                                 in1=xc[:, :T])
            stats_chain(runA[t], t, r1s[t][:, m, :], m)

        # pass 1: tile 0 alone; its LN1 + digit chain then overlaps pass 2.
        wts_a = {}
        for m in range(EO):
            wt = wp.tile([P, EO, 2, P], F8, tag="w16")
            if m == 0:
                for q in range(4):
                    nc.sync.dma_start(wt[:, 4 * q:4 * q + 4, :, :],
                                      wkv_d[:, m, 4 * q:4 * q + 4, :, :])
            else:
                nc.sync.dma_start(wt[:], wkv_d[:, m])
            a_block(0, m, wt)
            if m == 6:
                load_xd(1)
            if m == 10:
                load_xd(2)
            if m >= 12:
                pwt = wp.tile([P, EO, 2, P], F8, tag="w16", name="pwt")
                wts_a[m - 12] = pwt
                nc.sync.dma_start(pwt[:], wkv_d[:, m - 12])
        emit_ln1_digits(0)
        # pass 2: tiles 1,2 lockstep (t2 lagging one block); kv weights are
        # cheap enough to stream a second time (first two prefetched above).
        for mb in range(EO + 1):
            if 4 <= mb < EO:
                wt = wp.tile([P, EO, 2, P], F8, tag="w16")
                wts_a[mb] = wt
                nc.sync.dma_start(wt[:], wkv_d[:, mb])
            for t, off in ((1, 0), (2, 1)):
                m = mb - off
                if not (0 <= m < EO):
                    continue
                a_block(t, m, wts_a[m])
                if m == EO - 1:
                    emit_ln1_digits(t)
            drain(1)


        # ---------------- phase B1: fc matmul + gelu + u digits ----------
        uds = []
        for t in range(NT):
            ud = bigp.tile([P, NUR, TS[t]], F8, tag=f"big_{t}", name=f"ud{t}")
            uds.append(ud)
        FSKIP = (0, 8, 10)

        def fc_block(t, ma, wt):
                T = TS[t]
                ps = psmm.tile([P, 512], F32, tag="ps")
                hilo_group(ps, wt, hds[t], T)
                if ma < J2:
                    uf = f32t.tile([P, 512], F32, tag="f32")
                    nc.scalar.activation(uf[:, :T], ps[:, :T], AF.Gelu,
                                         bias=bfc_t[:, ma:ma + 1],
                                         scale=1.0 / PS)
                    nc.scalar.activation(uds[t][:, ma, :], uf[:, :T],
                                         AF.Identity, bias=negmu_t[:],
                                         scale=SA)
                    t3 = f32t.tile([P, 512], F32, tag="f32")
                    nc.vector.tensor_scalar(t3[:, :T], uf[:, :T], sa_t[:],
                                            negmu_t[:], ALU.mult, ALU.add)
                    nc.vector.tensor_tensor(uds[t][:, J2 + ma, :], t3[:, :T],
                                            uds[t][:, ma, :], ALU.subtract)
                else:
                    nc.scalar.activation(uds[t][:, 2 * J2 + (ma - J2), :],
                                         ps[:, :T], AF.Gelu,
                                         bias=bfc_t[:, ma:ma + 1],
                                         scale=1.0 / PS)

        for mb in range(FO):
            wt = wp.tile([P, EO, 2, P], F8, tag="w16")
            nc.sync.dma_start(wt[:], wfc_d[:, mb])
            for t in range(NT):
                if mb >= FSKIP[t]:
                    fc_block(t, mb, wt)
            drain(2)
        # catch-up: the first blocks t1/t2 skipped, with re-streamed weights
        for cb in range(max(FSKIP)):
            wt = wp.tile([P, EO, 2, P], F8, tag="w16")
            nc.sync.dma_start(wt[:], wfc_d[:, cb])
            for t in (1, 2):
                if cb < FSKIP[t]:
                    fc_block(t, cb, wt)

        # -------- phase B2 + C: mproj + LN2 + output, two stages --------
        # stage 2 (t1,t2) shares one 512-column v2f tile and one bc tile so
        # the exposed tail normalizes and stores both tiles full-width.
        v2fs = [None] * NT
        OFFV = (0, 0, TS[1])
        bc2s = [None] * NT
        outq = (nc.gpsimd, nc.sync)

        def phase_c_begin(t, runB):
            sB, qB = stats_mms(runB, t)
            bc2s[t] = ln_finalize(t, sB, qB, for_ln2=True)

        def phase_c_pair(t, g, tailq=False, wide=False):
            """normalize chunks 2g, 2g+1 of tile t in-place into the (dead)
            v2f rows; with wide=True the op covers the merged t1|t2 columns
            and stores go one-per-chunk across both tiles."""
            T = 512 if wide else TS[t]
            co = 0 if wide else OFFV[t]
            bc = bc2s[t]
            q = (nc.sync, nc.scalar, nc.gpsimd) if tailq else outq
            vsl = v2fs[t][:, 2 * g:2 * g + 2, co:co + T]
            tm = sqp.tile([P, 2, 512], BF16, tag="dig", bufs=3)
            nc.vector.tensor_mul(out=tm[:, :, :T], in0=vsl,
                                 in1=bc[:, 2:4, co:co + T])
            if LN2_TRIVIAL:
                nc.vector.tensor_tensor(vsl, tm[:, :, :T],
                                        bc[:, 0:2, co:co + T], ALU.subtract)
            else:
                for i in (0, 1):
                    m = 2 * g + i
                    t2 = sqp.tile([P, 512], BF16, tag="dg1")
                    nc.vector.tensor_tensor(t2[:, :T], tm[:, i, :T],
                                            bc[:, i, co:co + T], ALU.subtract)
                    nc.scalar.activation(v2fs[t][:, m, co:co + T], t2[:, :T],
                                         AF.Identity, bias=b2_t[:, m:m + 1],
                                         scale=g2_t[:, m:m + 1])
            cs = slice(TOFF[1], TOK) if wide else tsl(t)
            for i in (0, 1):
                m = 2 * g + i
                q[m % len(q)].dma_start(out_d[m * P:(m + 1) * P, cs],
                                        v2fs[t][:, m, co:co + T])

        def mp_pass_group(ps, wta, wtb, t, T):
            def cell(j):
                return wta[:, j, :, :] if j < EO else wtb[:, j - EO, :, :]

            for j in range(J2 // 2):
                nc.tensor.matmul(ps[:, :T], lhsT=cell(j),
                                 rhs=uds[t][:, 2 * j:2 * j + 2, :],
                                 start=(j == 0), stop=False, perf_mode=DRM)
                nc.tensor.matmul(ps[:, :T], lhsT=cell(j),
                                 rhs=uds[t][:, J2 + 2 * j:J2 + 2 * j + 2, :],
                                 start=False, stop=False, perf_mode=DRM)
            for p in range(JP // 2):
                nc.tensor.matmul(ps[:, :T], lhsT=cell(J2 // 2 + p),
                                 rhs=uds[t][:, 2 * J2 + 2 * p:
                                            2 * J2 + 2 * p + 2, :],
                                 start=False, stop=(p == JP // 2 - 1),
                                 perf_mode=DRM)

        prev_tiles = []
        prev_runB = {}
        for stage in ((0,), (1, 2)):
            runB = {t: dict() for t in stage}
            if stage == (0,):
                v2fs[0] = d32p.tile([P, EO, TS[0]], BF16, tag="d32_0",
                                    name="v2f0")
            else:
                v2f12 = bigp.tile([P, EO, 512], BF16, tag="big_0",
                                  name="v2f12")
                v2fs[1] = v2f12
                v2fs[2] = v2f12
            for mo in range(EO):
                wta = wp.tile([P, EO, 2, P], F8, tag="w16", name="wta")
                nc.sync.dma_start(wta[:], wmp_d[:, mo, :EO])
                wtb = wp.tile([P, EO, 2, P], F8, tag="w16", name="wtb")
                nc.sync.dma_start(wtb[:], wmp_d[:, mo, EO:])
                for t in stage:
                    T = TS[t]
                    co = OFFV[t]
                    ps = psmm.tile([P, 512], F32, tag="ps")
                    mp_pass_group(ps, wta, wtb, t, T)
                    xc = xbp.tile([P, 512], BF16, tag="xb")
                    nc.gpsimd.dma_start(xc[:, :T],
                                        xb_d[mo * P:(mo + 1) * P, tsl(t)])
                    tv = f32t.tile([P, 512], F32, tag="f32")
                    nc.scalar.activation(tv[:, :T], ps[:, :T], AF.Identity,
                                         bias=bmp_t[:, mo:mo + 1],
                                         scale=1.0 / PS)
                    vrow = v2fs[t][:, mo, co:co + T]
                    nc.vector.tensor_add(out=vrow, in0=tv[:, :T],
                                         in1=xc[:, :T])
                    stats_chain(runB[t], t, vrow, mo)
                for tp in prev_tiles:
                    if mo == 0:
                        phase_c_begin(tp, prev_runB[tp])
                    if mo % 2 == 0:
                        phase_c_pair(tp, mo // 2)

            prev_tiles = list(stage)
            prev_runB = runB
        # tail: LN2 + normalize + store for the merged t1|t2 tile, full-width
        bc12 = bcp.tile([P, 4, 512], BF16, tag="bc")
        for tp in prev_tiles:
            sB, qB = stats_mms(prev_runB[tp], tp)
            ln_finalize(tp, sB, qB, bc_out=bc12, coff=OFFV[tp],
                        for_ln2=True)
            bc2s[tp] = bc12
        for g in range(EO // 2):
            phase_c_pair(1, g, tailq=True, wide=True)

    nc.compile()
    return nc


def _get_nc(ln2_trivial=True):
    if ln2_trivial not in _CACHED_NC:
        _CACHED_NC[ln2_trivial] = _build(ln2_trivial)
    return _CACHED_NC[ln2_trivial]


def _q(x):
    return x.astype(E4NP)


def _dec(x):
    return x.astype(np.float32)


def _prep_inputs(x, w_kv, b_kv, w_fc, b_fc, w_mproj, b_mproj,
                 ln1_g, ln1_b, ln2_g, ln2_b):
    f32 = np.float32
    x_flat = np.ascontiguousarray(np.asarray(x, f32).reshape(B * S, E))

    def pack_hilo(W, KO, NO):
        """-> [P, NO, KO, 2, P] fp8; pair 2j=(Wh_2j,Wh_2j+1), 2j+1=lo pair."""
        Ws = (np.asarray(W, f32) * SW).reshape(KO, P, NO, P)
        Wh = _q(Ws)
        Wl = _q(Ws - _dec(Wh))
        Whp = Wh.reshape(KO // 2, 2, P, NO, P).transpose(3, 2, 0, 1, 4)
        Wlp = Wl.reshape(KO // 2, 2, P, NO, P).transpose(3, 2, 0, 1, 4)
        arr = np.stack([Whp, Wlp], axis=3)          # [NO,P,KO/2,2,2,P]
        arr = arr.reshape(NO, P, KO, 2, P).transpose(1, 0, 2, 3, 4)
        return np.ascontiguousarray(arr)            # [P,NO,KO,2,P]

    wkv_t = pack_hilo(w_kv, EO, EO)
    # fold LN1 affine into fc: z = (g1*hhat + b1) @ wfc + bfc
    #   = hhat @ (g1[:,None]*wfc) + (bfc + b1 @ wfc); digits quantize SA*hhat.
    g1 = np.asarray(ln1_g, f32)
    b1 = np.asarray(ln1_b, f32)
    wfc_f = np.asarray(w_fc, f32) * g1[:, None]
    bfc_f = np.asarray(b_fc, f32) + b1 @ np.asarray(w_fc, f32)
    wfc_t = pack_hilo(wfc_f, EO, FO)

    Wmp = np.asarray(w_mproj, f32).reshape(FO, P, EO, P)
    wc = Wmp[:J2] * SW
    w8c = _q(wc)
    wpl = Wmp[J2:] * (SA * SW)
    w8p = _q(wpl)
    pairs_c = _dec(w8c).reshape(J2 // 2, 2, P, EO, P).transpose(3, 2, 0, 1, 4)
    pairs_p = _dec(w8p).reshape(JP // 2, 2, P, EO, P).transpose(3, 2, 0, 1, 4)
    wmp_t = np.concatenate([_q(pairs_c), _q(pairs_p)], axis=2)
    wmp_t = np.ascontiguousarray(wmp_t.transpose(1, 0, 2, 3, 4))

    # host bias corrections (data-free)
    err_c = (wc - _dec(w8c)).sum((0, 1)) / SW            # [EO, P]
    err_p = (wpl - _dec(w8p)).sum((0, 1)) / (SA * SW)
    dec_c = _dec(w8c).sum((0, 1)) / SW
    bcorr = MU * (err_c + err_p) + MU * dec_c            # [EO, P]
    bmp_c = np.asarray(b_mproj, f32).reshape(EO, P) + bcorr

    def p2d(v):
        v = np.asarray(v, f32)
        return np.ascontiguousarray(v.reshape(-1, P).T)

    shared = {
        "wkv": wkv_t, "wfc": wfc_t, "wmp": wmp_t,
        "bkv": p2d(b_kv), "bfc": p2d(bfc_f),
        "bmp": np.ascontiguousarray(bmp_c.T),
        "g2": p2d(ln2_g), "b2": p2d(ln2_b),
    }
    in_maps = []
    for c in range(NCORES):
        xT = np.ascontiguousarray(x_flat[c * TOK:(c + 1) * TOK].T)  # [E, TOK]
        xs = xT * np.float32(SA)
        xh = _q(xs)
        xl = _q(xs - _dec(xh))
        xh = xh.reshape(EO, P, TOK)
        xl = xl.reshape(EO, P, TOK)
        # group-of-4 rows: [Ah_2g, Ah_2g+1, Al_2g, Al_2g+1]
        xd = np.empty((32, P, TOK), E4NP)
        for g in range(8):
            xd[4 * g] = xh[2 * g]
            xd[4 * g + 1] = xh[2 * g + 1]
            xd[4 * g + 2] = xl[2 * g]
            xd[4 * g + 3] = xl[2 * g + 1]
        in_maps.append({
            "xb": xT.astype(ml_dtypes.bfloat16),
            "xd": np.ascontiguousarray(xd.transpose(1, 0, 2)),
            **shared})
    return in_maps


def _run(inputs, trace=False):
    ln2_trivial = bool(np.all(np.asarray(inputs["ln2_g"]) == 1.0)
                       and np.all(np.asarray(inputs["ln2_b"]) == 0.0))
    nc = _get_nc(ln2_trivial)
    in_maps = _prep_inputs(
        inputs["x"], inputs["w_kv"], inputs["b_kv"], inputs["w_fc"],
        inputs["b_fc"], inputs["w_mproj"], inputs["b_mproj"],
        inputs["ln1_g"], inputs["ln1_b"], inputs["ln2_g"], inputs["ln2_b"])
    res = run_bass_kernel_spmd(nc, in_maps, core_ids=list(range(NCORES)),
                               trace=trace)
    outs = [np.asarray(res.results[c]["out"]).astype(np.float32).T
            for c in range(NCORES)]
    full = np.concatenate(outs, axis=0).reshape(B, S, E)
    return full, res


def kernel(**inputs) -> np.ndarray:
    out, _ = _run(inputs, trace=False)
    return out


# revision 5
# speedup vs baseline: 1.0146x; 1.0100x over previous
"""Trainium2 Bass kernel for nn_Block_68719476955 — all-fp8-DoubleRow version.

Math: with H=1 the attention softmax is over a singleton axis, so the whole
attention reduces to x @ w_kv + b_kv.

All three matmuls run as fp8-e4m3 DoubleRow pair-passes with digit
compensation:
  kv, fc: every k-chunk pair (c,c') gets 3 passes computing
      (Ah+Al)@Wh + Ah@Wl ~= A@W   (~bf16 accuracy, 0.75x bf16 pass count)
  mproj: first J2=54 k-chunks activation-compensated (2 passes per pair,
    rows are hi/lo digits of SA*(u-MU), cells W8 reused); last 10 chunks
    plain. Host folds MU*colsum(dec(W8)) (shift) and MU*colsum(Werr)
    (mu-trick, data-free) into b_mproj.

Distribution: data-parallel, 1024 tokens/core, token tiles (512, 384, 128)
lockstep per weight block for kv+LN1 and fc; mproj in two stages
(t0 alone, then t1+t2) so each stage's LN2+normalize+store hides under the
next stage's matmuls. Residual x streamed as bf16; weights partition-major
in DRAM (4KB contiguous per-partition rows) for full-rate DMA.

Emulated end-to-end rel err (emul4.py, J2=56): 1.938e-2 vs the 2e-2 gate.
"""

import numpy as np
import ml_dtypes
from contextlib import ExitStack

import concourse.bacc as bacc
import concourse.mybir as mybir
import concourse.tile as tile
from concourse.bass_utils import run_bass_kernel_spmd

P = 128
B, S, E = 4, 2048, 2048
H4 = 4 * E
NCORES = 8
TOK = (B * S) // NCORES    # 1024 tokens per core
TS = (512, 384, 128)       # token tiles
TOFF = (0, 512, 896)
NT = 3
EO = E // P                # 16
FO = H4 // P               # 64
LN_EPS = 1e-5
SA = 8.0                   # activation digit scale
SW = 64.0                  # weight digit scale
PS = SA * SW               # product scale of every DR slot
MU = 0.2423                # u-shift / mu-trick constant (design param)
J2 = 56                    # mproj k-chunks with activation compensation
JP = FO - J2               # plain mproj k-chunks (10)
NUR = 2 * J2 + JP          # u digit rows (118)
NWMP = J2 // 2 + JP // 2   # mproj weight pairs per block (32)

F32 = mybir.dt.float32
BF16 = mybir.dt.bfloat16
F8 = mybir.dt.float8e4
DRM = mybir.MatmulPerfMode.DoubleRow
AF = mybir.ActivationFunctionType
ALU = mybir.AluOpType
E4NP = ml_dtypes.float8_e4m3

_CACHED_NC = {}


def _build(ln2_trivial):
    LN2_TRIVIAL = ln2_trivial
    nc = bacc.Bacc(None, target_bir_lowering=False)

    xb_d = nc.dram_tensor("xb", [E, TOK], BF16, kind="ExternalInput")
    xd_d = nc.dram_tensor("xd", [P, 32, TOK], F8, kind="ExternalInput")
    wkv_d = nc.dram_tensor("wkv", [P, EO, EO, 2, P], F8, kind="ExternalInput")
    wfc_d = nc.dram_tensor("wfc", [P, FO, EO, 2, P], F8, kind="ExternalInput")
    wmp_d = nc.dram_tensor("wmp", [P, EO, NWMP, 2, P], F8, kind="ExternalInput")
    bkv_d = nc.dram_tensor("bkv", [P, EO], F32, kind="ExternalInput")
    bfc_d = nc.dram_tensor("bfc", [P, FO], F32, kind="ExternalInput")
    bmp_d = nc.dram_tensor("bmp", [P, EO], F32, kind="ExternalInput")
    g2_d = nc.dram_tensor("g2", [P, EO], F32, kind="ExternalInput")
    b2_d = nc.dram_tensor("b2", [P, EO], F32, kind="ExternalInput")
    out_d = nc.dram_tensor("out", [E, TOK], BF16, kind="ExternalOutput")

    with tile.TileContext(nc) as tc, ExitStack() as ctx:
        consts = ctx.enter_context(tc.tile_pool(name="consts", bufs=1))
        # 32-row fp8 arenas per tile: x digits -> h digits -> (as bf16) v+x
        d32p = ctx.enter_context(tc.tile_pool(name="d32p", bufs=1))
        # big arena per tile: r1 (f32, 16 rows) -> u digits (fp8, 118 rows)
        bigp = ctx.enter_context(tc.tile_pool(name="bigp", bufs=1))
        wp = ctx.enter_context(tc.tile_pool(name="wp", bufs=5))
        f32t = ctx.enter_context(tc.tile_pool(name="f32t", bufs=3))
        xbp = ctx.enter_context(tc.tile_pool(name="xbp", bufs=3))
        sqp = ctx.enter_context(tc.tile_pool(name="sqp", bufs=4))
        accp = ctx.enter_context(tc.tile_pool(name="accp", bufs=1))
        stp = ctx.enter_context(tc.tile_pool(name="stp", bufs=2))
        bcp = ctx.enter_context(tc.tile_pool(name="bcp", bufs=1))
        psmm = ctx.enter_context(tc.tile_pool(name="psmm", bufs=4, space="PSUM"))
        psst = ctx.enter_context(tc.tile_pool(name="psst", bufs=2, space="PSUM"))

        def tsl(t):
            return slice(TOFF[t], TOFF[t] + TS[t])

        # ---- constants (gpsimd queue) ----
        bkv_t = consts.tile([P, EO], F32)
        nc.gpsimd.dma_start(bkv_t[:], bkv_d[:, :])
        bfc_t = consts.tile([P, FO], F32)
        nc.gpsimd.dma_start(bfc_t[:], bfc_d[:, :])
        bmp_t = consts.tile([P, EO], F32)
        nc.gpsimd.dma_start(bmp_t[:], bmp_d[:, :])
        g2_t = consts.tile([P, EO], F32)
        nc.gpsimd.dma_start(g2_t[:], g2_d[:, :])
        b2_t = consts.tile([P, EO], F32)
        nc.gpsimd.dma_start(b2_t[:], b2_d[:, :])
        ones_col = consts.tile([P, 1], BF16)
        nc.vector.memset(ones_col[:], 1.0 / E)
        eps_t = consts.tile([1, 1], F32)
        nc.vector.memset(eps_t[:], LN_EPS / (SA * SA))
        eps2_t = consts.tile([1, 1], F32)
        nc.vector.memset(eps2_t[:], LN_EPS)
        negmu_t = consts.tile([P, 1], F32)
        nc.vector.memset(negmu_t[:], -MU * SA)
        sa_t = consts.tile([P, 1], F32)
        nc.vector.memset(sa_t[:], SA)

        # ---- x digit tiles: groups of 4 rows (Ah,Ah',Al,Al') per pair ----
        xds = []
        for t in range(NT):
            xd = d32p.tile([P, 32, TS[t]], F8, tag=f"d32_{t}", name=f"xd{t}")
            xds.append(xd)
        dmaq = (nc.scalar, nc.gpsimd)

        def load_xd(t):
            for g in range(8):
                dmaq[g % 2].dma_start(xds[t][:, 4 * g:4 * g + 4, :],
                                      xd_d[:, 4 * g:4 * g + 4, tsl(t)])

        load_xd(0)

        # warm MMs: keep PE busy while the first DMAs land (p-state ramp)
        warm_rhs = consts.tile([P, 512], BF16)
        nc.vector.memset(warm_rhs[:], 1.0)
        warm_ps = psst.tile([1, 512], F32, tag="pss")
        for _ in range(14):
            nc.tensor.matmul(warm_ps[:], lhsT=ones_col[:],
                             rhs=warm_rhs[:], start=True, stop=True)

        def hilo_group(ps, wt, rows, T):
            """24 DR passes: (Ah+Al)@Wh + Ah@Wl for 8 chunk pairs."""
            for g in range(8):
                hi = rows[:, 4 * g:4 * g + 2, :]
                lo = rows[:, 4 * g + 2:4 * g + 4, :]
                nc.tensor.matmul(ps[:, :T], lhsT=wt[:, 2 * g, :, :], rhs=hi,
                                 start=(g == 0), stop=False, perf_mode=DRM)
                nc.tensor.matmul(ps[:, :T], lhsT=wt[:, 2 * g, :, :], rhs=lo,
                                 start=False, stop=False, perf_mode=DRM)
                nc.tensor.matmul(ps[:, :T], lhsT=wt[:, 2 * g + 1, :, :], rhs=hi,
                                 start=False, stop=(g == 7), perf_mode=DRM)

        def ln_finalize(t, ps_sum, ps_sq, bc_out=None, coff=0,
                        for_ln2=False, defer_bcast=False):
            """bc [P,4,T] bf16: rows (m*rstd*SA, m*rstd*SA, rstd*SA, rstd*SA)
            so chunk-PAIR ops can slice bc[:,0:2] / bc[:,2:4] directly.
            ps_sum/ps_sq arrive pre-divided by E (ones = 1/E)."""
            T = TS[t]
            st = stp.tile([1, 4, 512], BF16, tag="st")
            nc.vector.tensor_scalar_mul(st[:, 1, :T], ps_sum[:, :T], 1.0)
            nc.vector.tensor_mul(out=st[:, 3, :T], in0=st[:, 1, :T],
                                 in1=st[:, 1, :T])
            nc.vector.tensor_tensor(st[:, 2, :T], ps_sq[:, :T], st[:, 3, :T],
                                    ALU.subtract)
            if for_ln2:
                nc.scalar.activation(st[:, 3, :T], st[:, 2, :T], AF.Sqrt,
                                     bias=eps2_t[:], scale=1.0)
            else:
                nc.scalar.activation(st[:, 3, :T], st[:, 2, :T], AF.Sqrt,
                                     bias=eps_t[:], scale=1.0 / (SA * SA))
            with nc.allow_low_precision(reason="bf16 rstd: ~0.2% scale "
                                        "error, well inside the fp8 budget"):
                nc.vector.reciprocal(out=st[:, 2, :T], in_=st[:, 3, :T])
            nc.vector.tensor_scalar_mul(st[:, 3, :T], st[:, 2, :T], 1.0)
            nc.vector.tensor_mul(out=st[:, 0, :T], in0=st[:, 1, :T],
                                 in1=st[:, 2, :T])
            nc.vector.tensor_scalar_mul(st[:, 1, :T], st[:, 0, :T], 1.0)
            if defer_bcast:
                return st
            bc = bc_out
            if bc is None:
                bc = bcp.tile([P, 4, 512], BF16, tag="bc")
            # rstd rows first: the chunk-pair muls only need bc[:,2:4]
            nc.gpsimd.partition_broadcast(bc[:, 2:4, coff:coff + T],
                                          st[:, 2:4, :T])
            nc.gpsimd.partition_broadcast(bc[:, 0:2, coff:coff + T],
                                          st[:, 0:2, :T])
            return bc

        def stats_chain(run, t, val_f32, m):
            """bf16 running sums of values (DVE) and squares (ACT + DVE)."""
            T = TS[t]
            if m == 0:
                pr = accp.tile([P, TS[t]], BF16, tag=f"pr{t}")
                nc.vector.tensor_scalar_mul(pr[:, :T], val_f32, 1.0)
                pq = accp.tile([P, TS[t]], BF16, tag=f"pq{t}")
                nc.scalar.activation(pq[:, :T], val_f32, AF.Square)
                run["pr"], run["pq"] = pr, pq
            else:
                sq = sqp.tile([P, 512], BF16, tag="sq", bufs=3)
                nc.scalar.activation(sq[:, :T], val_f32, AF.Square)
                nc.vector.tensor_tensor(run["pr"][:, :T], run["pr"][:, :T],
                                        val_f32, ALU.add)
                nc.vector.tensor_tensor(run["pq"][:, :T], run["pq"][:, :T],
                                        sq[:, :T], ALU.add)

        def stats_mms(run, t):
            T = TS[t]
            ps_sum = psst.tile([1, 512], F32, tag="pss")
            ps_sq = psst.tile([1, 512], F32, tag="psq")
            nc.tensor.matmul(ps_sum[:, :T], lhsT=ones_col[:],
                             rhs=run["pr"][:, :T], start=True, stop=True)
            nc.tensor.matmul(ps_sq[:, :T], lhsT=ones_col[:],
                             rhs=run["pq"][:, :T], start=True, stop=True)
            return ps_sum, ps_sq

        # ---------------- LN1 finalize + h digits (per tile) -------------
        # h digit rows mirror the x layout: chunk c -> hi row 4*(c//2)+(c%2),
        # lo row = hi row + 2. LN1's gamma/beta are folded into wfc/bfc on the
        # host, so digits quantize the bare normalized SA*h (bf16 chain, 2x).
        hds = [None] * NT
        pending = []   # deferred digit-pair closures, drained 1-3 per block

        def drain(n):
            for _ in range(min(n, len(pending))):
                pending.pop(0)()

        def emit_ln1_digits(t):
            sA, qA = stats_mms(runA[t], t)
            bc = ln_finalize(t, sA, qA)
            T = TS[t]
            hd = d32p.tile([P, 32, T], F8, tag=f"d32_{t}", name=f"hd{t}")
            hds[t] = hd

            def digit_pair(g, t=t, bc=bc, hd=hd, T=T):
                tm = sqp.tile([P, 2, 512], BF16, tag="dig", bufs=3)
                nc.vector.tensor_mul(out=tm[:, :, :T],
                                     in0=r1s[t][:, 2 * g:2 * g + 2, :],
                                     in1=bc[:, 2:4, :T])
                t2 = sqp.tile([P, 2, 512], BF16, tag="dig", bufs=3)
                nc.vector.tensor_tensor(t2[:, :, :T], tm[:, :, :T],
                                        bc[:, 0:2, :T], ALU.subtract)
                nc.scalar.activation(hd[:, 4 * g:4 * g + 2, :], t2[:, :, :T],
                                     AF.Identity)
                nc.vector.tensor_tensor(hd[:, 4 * g + 2:4 * g + 4, :],
                                        t2[:, :, :T],
                                        hd[:, 4 * g:4 * g + 2, :],
                                        ALU.subtract)
            for g in range(EO // 2):
                pending.append(lambda g=g: digit_pair(g))

        # ---------------- phase A: kv matmul + residual ----------------
        # tile offsets (0,1,2): tile t processes block mb - t, so t0 finishes
        # early and its LN1/digit chain overlaps the other tiles' tail blocks.
        r1s, runA = [], [dict() for _ in range(NT)]
        for t in range(NT):
            r1 = bigp.tile([P, EO, TS[t]], BF16, tag=f"big_{t}", name=f"r1{t}")
            r1s.append(r1)
        def a_block(t, m, wt):
            T = TS[t]
            ps = psmm.tile([P, 512], F32, tag="ps")
            hilo_group(ps, wt, xds[t], T)
            xc = xbp.tile([P, 512], BF16, tag="xb")
            (nc.sync if t == 0 else nc.gpsimd).dma_start(
                xc[:, :T], xb_d[m * P:(m + 1) * P, tsl(t)])
            t1 = f32t.tile([P, 512], F32, tag="f32")
            nc.scalar.activation(t1[:, :T], ps[:, :T], AF.Identity,
                                 bias=bkv_t[:, m:m + 1], scale=1.0 / PS)
            nc.vector.tensor_add(out=r1s[t][:, m, :], in0=t1[:, :T],
                                 in1=xc[:, :T])
            stats_chain(runA[t], t, r1s[t][:, m, :], m)

        # pass 1: tile 0 alone; its LN1 + digit chain then overlaps pass 2.
        wts_a = {}
        for m in range(EO):
            wt = wp.tile([P, EO, 2, P], F8, tag="w16")
            if m == 0:
                for q in range(4):
                    nc.sync.dma_start(wt[:, 4 * q:4 * q + 4, :, :],
                                      wkv_d[:, m, 4 * q:4 * q + 4, :, :])
            else:
                nc.sync.dma_start(wt[:], wkv_d[:, m])
            a_block(0, m, wt)
            if m == 6:
                load_xd(1)
            if m == 10:
                load_xd(2)
            if m >= 12:
                pwt = wp.tile([P, EO, 2, P], F8, tag="w16", name="pwt")
                wts_a[m - 12] = pwt
                nc.sync.dma_start(pwt[:], wkv_d[:, m - 12])
        emit_ln1_digits(0)
        # pass 2: tiles 1,2 lockstep (t2 lagging one block); kv weights are
        # cheap enough to stream a second time (first two prefetched above).
        for mb in range(EO + 1):
            if 4 <= mb < EO:
                wt = wp.tile([P, EO, 2, P], F8, tag="w16")
                wts_a[mb] = wt
                nc.sync.dma_start(wt[:], wkv_d[:, mb])
            for t, off in ((1, 0), (2, 1)):
                m = mb - off
                if not (0 <= m < EO):
                    continue
                a_block(t, m, wts_a[m])
                if m == EO - 1:
                    emit_ln1_digits(t)
            drain(1)


        # ---------------- phase B1: fc matmul + gelu + u digits ----------
        uds = []
        for t in range(NT):
            ud = bigp.tile([P, NUR, TS[t]], F8, tag=f"big_{t}", name=f"ud{t}")
            uds.append(ud)
        FSKIP = (0, 8, 10)

        def fc_block(t, ma, wt):
                T = TS[t]
                ps = psmm.tile([P, 512], F32, tag="ps")
                hilo_group(ps, wt, hds[t], T)
                if ma < J2:
                    uf = f32t.tile([P, 512], F32, tag="f32")
                    nc.scalar.activation(uf[:, :T], ps[:, :T], AF.Gelu,
                                         bias=bfc_t[:, ma:ma + 1],
                                         scale=1.0 / PS)
                    nc.scalar.activation(uds[t][:, ma, :], uf[:, :T],
                                         AF.Identity, bias=negmu_t[:],
                                         scale=SA)
                    t3 = f32t.tile([P, 512], F32, tag="f32")
                    nc.vector.tensor_scalar(t3[:, :T], uf[:, :T], sa_t[:],
                                            negmu_t[:], ALU.mult, ALU.add)
                    nc.vector.tensor_tensor(uds[t][:, J2 + ma, :], t3[:, :T],
                                            uds[t][:, ma, :], ALU.subtract)
                else:
                    nc.scalar.activation(uds[t][:, 2 * J2 + (ma - J2), :],
                                         ps[:, :T], AF.Gelu,
                                         bias=bfc_t[:, ma:ma + 1],
                                         scale=1.0 / PS)

        for mb in range(FO):
            wt = wp.tile([P, EO, 2, P], F8, tag="w16")
            nc.sync.dma_start(wt[:], wfc_d[:, mb])
            for t in range(NT):
                if mb >= FSKIP[t]:
                    fc_block(t, mb, wt)
            drain(2)
        # catch-up: the first blocks t1/t2 skipped, with re-streamed weights
        for cb in range(max(FSKIP)):
            wt = wp.tile([P, EO, 2, P], F8, tag="w16")
            nc.sync.dma_start(wt[:], wfc_d[:, cb])
            for t in (1, 2):
                if cb < FSKIP[t]:
                    fc_block(t, cb, wt)

        # -------- phase B2 + C: mproj + LN2 + output, two stages --------
        # stage 2 (t1,t2) shares one 512-column v2f tile and one bc tile so
        # the exposed tail normalizes and stores both tiles full-width.
        v2fs = [None] * NT
        OFFV = (0, 0, TS[1])
        bc2s = [None] * NT
        outq = (nc.gpsimd, nc.sync)

        def phase_c_begin(t, runB):
            sB, qB = stats_mms(runB, t)
            bc2s[t] = ln_finalize(t, sB, qB, for_ln2=True)

        def phase_c_pair(t, g, tailq=False, wide=False):
            """normalize chunks 2g, 2g+1 of tile t in-place into the (dead)
            v2f rows; with wide=True the op covers the merged t1|t2 columns
            and stores go one-per-chunk across both tiles."""
            T = 512 if wide else TS[t]
            co = 0 if wide else OFFV[t]
            bc = bc2s[t]
            q = (nc.sync, nc.scalar, nc.gpsimd) if tailq else outq
            vsl = v2fs[t][:, 2 * g:2 * g + 2, co:co + T]
            tm = sqp.tile([P, 2, 512], BF16, tag="dig", bufs=3)
            nc.vector.tensor_mul(out=tm[:, :, :T], in0=vsl,
                                 in1=bc[:, 2:4, co:co + T])
            if LN2_TRIVIAL:
                nc.vector.tensor_tensor(vsl, tm[:, :, :T],
                                        bc[:, 0:2, co:co + T], ALU.subtract)
            else:
                for i in (0, 1):
                    m = 2 * g + i
                    t2 = sqp.tile([P, 512], BF16, tag="dg1")
                    nc.vector.tensor_tensor(t2[:, :T], tm[:, i, :T],
                                            bc[:, i, co:co + T], ALU.subtract)
                    nc.scalar.activation(v2fs[t][:, m, co:co + T], t2[:, :T],
                                         AF.Identity, bias=b2_t[:, m:m + 1],
                                         scale=g2_t[:, m:m + 1])
            cs = slice(TOFF[1], TOK) if wide else tsl(t)
            for i in (0, 1):
                m = 2 * g + i
                q[m % len(q)].dma_start(out_d[m * P:(m + 1) * P, cs],
                                        v2fs[t][:, m, co:co + T])

        def mp_pass_group(ps, wta, wtb, t, T):
            def cell(j):
                return wta[:, j, :, :] if j < EO else wtb[:, j - EO, :, :]

            for j in range(J2 // 2):
                nc.tensor.matmul(ps[:, :T], lhsT=cell(j),
                                 rhs=uds[t][:, 2 * j:2 * j + 2, :],
                                 start=(j == 0), stop=False, perf_mode=DRM)
                nc.tensor.matmul(ps[:, :T], lhsT=cell(j),
                                 rhs=uds[t][:, J2 + 2 * j:J2 + 2 * j + 2, :],
                                 start=False, stop=False, perf_mode=DRM)
            for p in range(JP // 2):
                nc.tensor.matmul(ps[:, :T], lhsT=cell(J2 // 2 + p),
                                 rhs=uds[t][:, 2 * J2 + 2 * p:
                                            2 * J2 + 2 * p + 2, :],
                                 start=False, stop=(p == JP // 2 - 1),
                                 perf_mode=DRM)

        prev_tiles = []
        prev_runB = {}
        for stage in ((0,), (1, 2)):
            runB = {t: dict() for t in stage}
            if stage == (0,):
                v2fs[0] = d32p.tile([P, EO, TS[0]], BF16, tag="d32_0",
                                    name="v2f0")
            else:
                v2f12 = bigp.tile([P, EO, 512], BF16, tag="big_0",
                                  name="v2f12")
                v2fs[1] = v2f12
                v2fs[2] = v2f12
            for mo in range(EO):
                wta = wp.tile([P, EO, 2, P], F8, tag="w16", name="wta")
                nc.sync.dma_start(wta[:], wmp_d[:, mo, :EO])
                wtb = wp.tile([P, EO, 2, P], F8, tag="w16", name="wtb")
                nc.sync.dma_start(wtb[:], wmp_d[:, mo, EO:])
                for t in stage:
                    T = TS[t]
                    co = OFFV[t]
                    ps = psmm.tile([P, 512], F32, tag="ps")
                    mp_pass_group(ps, wta, wtb, t, T)
                    xc = xbp.tile([P, 512], BF16, tag="xb")
                    nc.gpsimd.dma_start(xc[:, :T],
                                        xb_d[mo * P:(mo + 1) * P, tsl(t)])
                    tv = f32t.tile([P, 512], F32, tag="f32")
                    nc.scalar.activation(tv[:, :T], ps[:, :T], AF.Identity,
                                         bias=bmp_t[:, mo:mo + 1],
                                         scale=1.0 / PS)
                    vrow = v2fs[t][:, mo, co:co + T]
                    nc.vector.tensor_add(out=vrow, in0=tv[:, :T],
                                         in1=xc[:, :T])
                    stats_chain(runB[t], t, vrow, mo)
                for tp in prev_tiles:
                    if mo == 0:
                        phase_c_begin(tp, prev_runB[tp])
                    if mo % 2 == 0:
                        phase_c_pair(tp, mo // 2)

            prev_tiles = list(stage)
            prev_runB = runB
        # tail: LN2 + normalize + store for the merged t1|t2 tile. Both
        # tiles' rstd rows broadcast before the mean rows so the first wide
        # mul is gated on as little as possible; pair ops run in place.
        bc12 = bcp.tile([P, 4, 512], BF16, tag="bc")
        sts = {}
        for tp in prev_tiles:
            sB, qB = stats_mms(prev_runB[tp], tp)
            sts[tp] = ln_finalize(tp, sB, qB, for_ln2=True, defer_bcast=True)
            bc2s[tp] = bc12
        for rows in ((2, 4), (0, 2)):
            for tp in prev_tiles:
                Tt = TS[tp]
                nc.gpsimd.partition_broadcast(
                    bc12[:, rows[0]:rows[1], OFFV[tp]:OFFV[tp] + Tt],
                    sts[tp][:, rows[0]:rows[1], :Tt])
        tq = (nc.sync, nc.scalar, nc.gpsimd)
        if LN2_TRIVIAL:
            for g in range(EO // 2):
                vsl = v2f12[:, 2 * g:2 * g + 2, :]
                nc.vector.tensor_mul(out=vsl, in0=vsl, in1=bc12[:, 2:4, :])
                nc.vector.tensor_tensor(vsl, vsl, bc12[:, 0:2, :],
                                        ALU.subtract)
                for i in (0, 1):
                    m = 2 * g + i
                    tq[m % 3].dma_start(out_d[m * P:(m + 1) * P,
                                              slice(TOFF[1], TOK)],
                                        v2f12[:, m, :])
        else:
            for g in range(EO // 2):
                phase_c_pair(1, g, tailq=True, wide=True)

    nc.compile()
    return nc


def _get_nc(ln2_trivial=True):
    if ln2_trivial not in _CACHED_NC:
        _CACHED_NC[ln2_trivial] = _build(ln2_trivial)
    return _CACHED_NC[ln2_trivial]


def _q(x):
    return x.astype(E4NP)


def _dec(x):
    return x.astype(np.float32)


def _prep_inputs(x, w_kv, b_kv, w_fc, b_fc, w_mproj, b_mproj,
                 ln1_g, ln1_b, ln2_g, ln2_b):
    f32 = np.float32
    x_flat = np.ascontiguousarray(np.asarray(x, f32).reshape(B * S, E))

    def pack_hilo(W, KO, NO):
        """-> [P, NO, KO, 2, P] fp8; pair 2j=(Wh_2j,Wh_2j+1), 2j+1=lo pair."""
        Ws = (np.asarray(W, f32) * SW).reshape(KO, P, NO, P)
        Wh = _q(Ws)
        Wl = _q(Ws - _dec(Wh))
        Whp = Wh.reshape(KO // 2, 2, P, NO, P).transpose(3, 2, 0, 1, 4)
        Wlp = Wl.reshape(KO // 2, 2, P, NO, P).transpose(3, 2, 0, 1, 4)
        arr = np.stack([Whp, Wlp], axis=3)          # [NO,P,KO/2,2,2,P]
        arr = arr.reshape(NO, P, KO, 2, P).transpose(1, 0, 2, 3, 4)
        return np.ascontiguousarray(arr)            # [P,NO,KO,2,P]

    wkv_t = pack_hilo(w_kv, EO, EO)
    # fold LN1 affine into fc: z = (g1*hhat + b1) @ wfc + bfc
    #   = hhat @ (g1[:,None]*wfc) + (bfc + b1 @ wfc); digits quantize SA*hhat.
    g1 = np.asarray(ln1_g, f32)
    b1 = np.asarray(ln1_b, f32)
    wfc_f = np.asarray(w_fc, f32) * g1[:, None]
    bfc_f = np.asarray(b_fc, f32) + b1 @ np.asarray(w_fc, f32)
    wfc_t = pack_hilo(wfc_f, EO, FO)

    Wmp = np.asarray(w_mproj, f32).reshape(FO, P, EO, P)
    wc = Wmp[:J2] * SW
    w8c = _q(wc)
    wpl = Wmp[J2:] * (SA * SW)
    w8p = _q(wpl)
    pairs_c = _dec(w8c).reshape(J2 // 2, 2, P, EO, P).transpose(3, 2, 0, 1, 4)
    pairs_p = _dec(w8p).reshape(JP // 2, 2, P, EO, P).transpose(3, 2, 0, 1, 4)
    wmp_t = np.concatenate([_q(pairs_c), _q(pairs_p)], axis=2)
    wmp_t = np.ascontiguousarray(wmp_t.transpose(1, 0, 2, 3, 4))

    # host bias corrections (data-free)
    err_c = (wc - _dec(w8c)).sum((0, 1)) / SW            # [EO, P]
    err_p = (wpl - _dec(w8p)).sum((0, 1)) / (SA * SW)
    dec_c = _dec(w8c).sum((0, 1)) / SW
    bcorr = MU * (err_c + err_p) + MU * dec_c            # [EO, P]
    bmp_c = np.asarray(b_mproj, f32).reshape(EO, P) + bcorr

    def p2d(v):
        v = np.asarray(v, f32)
        return np.ascontiguousarray(v.reshape(-1, P).T)

    shared = {
        "wkv": wkv_t, "wfc": wfc_t, "wmp": wmp_t,
        "bkv": p2d(b_kv), "bfc": p2d(bfc_f),
        "bmp": np.ascontiguousarray(bmp_c.T),
        "g2": p2d(ln2_g), "b2": p2d(ln2_b),
    }
    in_maps = []
    for c in range(NCORES):
        xT = np.ascontiguousarray(x_flat[c * TOK:(c + 1) * TOK].T)  # [E, TOK]
        xs = xT * np.float32(SA)
        xh = _q(xs)
        xl = _q(xs - _dec(xh))
        xh = xh.reshape(EO, P, TOK)
        xl = xl.reshape(EO, P, TOK)
        # group-of-4 rows: [Ah_2g, Ah_2g+1, Al_2g, Al_2g+1]
        xd = np.empty((32, P, TOK), E4NP)
        for g in range(8):
            xd[4 * g] = xh[2 * g]
            xd[4 * g + 1] = xh[2 * g + 1]
            xd[4 * g + 2] = xl[2 * g]
            xd[4 * g + 3] = xl[2 * g + 1]
        in_maps.append({
            "xb": xT.astype(ml_dtypes.bfloat16),
            "xd": np.ascontiguousarray(xd.transpose(1, 0, 2)),
            **shared})
    return in_maps


def _run(inputs, trace=False):
    ln2_trivial = bool(np.all(np.asarray(inputs["ln2_g"]) == 1.0)
                       and np.all(np.asarray(inputs["ln2_b"]) == 0.0))
    nc = _get_nc(ln2_trivial)
    in_maps = _prep_inputs(
        inputs["x"], inputs["w_kv"], inputs["b_kv"], inputs["w_fc"],
        inputs["b_fc"], inputs["w_mproj"], inputs["b_mproj"],
        inputs["ln1_g"], inputs["ln1_b"], inputs["ln2_g"], inputs["ln2_b"])
    res = run_bass_kernel_spmd(nc, in_maps, core_ids=list(range(NCORES)),
                               trace=trace)
    outs = [np.asarray(res.results[c]["out"]).astype(np.float32).T
            for c in range(NCORES)]
    full = np.concatenate(outs, axis=0).reshape(B, S, E)
    return full, res


def kernel(**inputs) -> np.ndarray:
    out, _ = _run(inputs, trace=False)
    return out


# revision 6
# speedup vs baseline: 1.0172x; 1.0025x over previous
"""Trainium2 Bass kernel for nn_Block_68719476955 — all-fp8-DoubleRow version.

Math: with H=1 the attention softmax is over a singleton axis, so the whole
attention reduces to x @ w_kv + b_kv.

All three matmuls run as fp8-e4m3 DoubleRow pair-passes with digit
compensation:
  kv, fc: every k-chunk pair (c,c') gets 3 passes computing
      (Ah+Al)@Wh + Ah@Wl ~= A@W   (~bf16 accuracy, 0.75x bf16 pass count)
  mproj: first J2=54 k-chunks activation-compensated (2 passes per pair,
    rows are hi/lo digits of SA*(u-MU), cells W8 reused); last 10 chunks
    plain. Host folds MU*colsum(dec(W8)) (shift) and MU*colsum(Werr)
    (mu-trick, data-free) into b_mproj.

Distribution: data-parallel, 1024 tokens/core, token tiles (512, 384, 128)
lockstep per weight block for kv+LN1 and fc; mproj in two stages
(t0 alone, then t1+t2) so each stage's LN2+normalize+store hides under the
next stage's matmuls. Residual x streamed as bf16; weights partition-major
in DRAM (4KB contiguous per-partition rows) for full-rate DMA.

Emulated end-to-end rel err (emul4.py, J2=56): 1.938e-2 vs the 2e-2 gate.
"""

import numpy as np
import ml_dtypes
from contextlib import ExitStack

import concourse.bacc as bacc
import concourse.mybir as mybir
import concourse.tile as tile
from concourse.bass_utils import run_bass_kernel_spmd

P = 128
B, S, E = 4, 2048, 2048
H4 = 4 * E
NCORES = 8
TOK = (B * S) // NCORES    # 1024 tokens per core
TS = (512, 384, 128)       # token tiles
TOFF = (0, 512, 896)
NT = 3
EO = E // P                # 16
FO = H4 // P               # 64
LN_EPS = 1e-5
SA = 8.0                   # activation digit scale
SW = 64.0                  # weight digit scale
PS = SA * SW               # product scale of every DR slot
MU = 0.2423                # u-shift / mu-trick constant (design param)
J2 = 56                    # mproj k-chunks with activation compensation
JP = FO - J2               # plain mproj k-chunks (10)
NUR = 2 * J2 + JP          # u digit rows (118)
NWMP = J2 // 2 + JP // 2   # mproj weight pairs per block (32)

F32 = mybir.dt.float32
BF16 = mybir.dt.bfloat16
F8 = mybir.dt.float8e4
DRM = mybir.MatmulPerfMode.DoubleRow
AF = mybir.ActivationFunctionType
ALU = mybir.AluOpType
E4NP = ml_dtypes.float8_e4m3

_CACHED_NC = {}


def _build(ln2_trivial):
    LN2_TRIVIAL = ln2_trivial
    nc = bacc.Bacc(None, target_bir_lowering=False)

    xb_d = nc.dram_tensor("xb", [E, TOK], BF16, kind="ExternalInput")
    xd_d = nc.dram_tensor("xd", [P, 32, TOK], F8, kind="ExternalInput")
    wkv_d = nc.dram_tensor("wkv", [P, EO, EO, 2, P], F8, kind="ExternalInput")
    wfc_d = nc.dram_tensor("wfc", [P, FO, EO, 2, P], F8, kind="ExternalInput")
    wmp_d = nc.dram_tensor("wmp", [P, EO, NWMP, 2, P], F8, kind="ExternalInput")
    bkv_d = nc.dram_tensor("bkv", [P, EO], F32, kind="ExternalInput")
    bfc_d = nc.dram_tensor("bfc", [P, FO], F32, kind="ExternalInput")
    bmp_d = nc.dram_tensor("bmp", [P, EO], F32, kind="ExternalInput")
    g2_d = nc.dram_tensor("g2", [P, EO], F32, kind="ExternalInput")
    b2_d = nc.dram_tensor("b2", [P, EO], F32, kind="ExternalInput")
    out_d = nc.dram_tensor("out", [E, TOK], BF16, kind="ExternalOutput")

    with tile.TileContext(nc) as tc, ExitStack() as ctx:
        consts = ctx.enter_context(tc.tile_pool(name="consts", bufs=1))
        # 32-row fp8 arenas per tile: x digits -> h digits -> (as bf16) v+x
        d32p = ctx.enter_context(tc.tile_pool(name="d32p", bufs=1))
        # big arena per tile: r1 (f32, 16 rows) -> u digits (fp8, 118 rows)
        bigp = ctx.enter_context(tc.tile_pool(name="bigp", bufs=1))
        wp = ctx.enter_context(tc.tile_pool(name="wp", bufs=5))
        f32t = ctx.enter_context(tc.tile_pool(name="f32t", bufs=3))
        xbp = ctx.enter_context(tc.tile_pool(name="xbp", bufs=3))
        sqp = ctx.enter_context(tc.tile_pool(name="sqp", bufs=4))
        accp = ctx.enter_context(tc.tile_pool(name="accp", bufs=1))
        stp = ctx.enter_context(tc.tile_pool(name="stp", bufs=2))
        bcp = ctx.enter_context(tc.tile_pool(name="bcp", bufs=1))
        psmm = ctx.enter_context(tc.tile_pool(name="psmm", bufs=4, space="PSUM"))
        psst = ctx.enter_context(tc.tile_pool(name="psst", bufs=2, space="PSUM"))

        def tsl(t):
            return slice(TOFF[t], TOFF[t] + TS[t])

        # ---- constants (gpsimd queue) ----
        bkv_t = consts.tile([P, EO], F32)
        nc.gpsimd.dma_start(bkv_t[:], bkv_d[:, :])
        bfc_t = consts.tile([P, FO], F32)
        nc.gpsimd.dma_start(bfc_t[:], bfc_d[:, :])
        bmp_t = consts.tile([P, EO], F32)
        nc.gpsimd.dma_start(bmp_t[:], bmp_d[:, :])
        g2_t = consts.tile([P, EO], F32)
        nc.gpsimd.dma_start(g2_t[:], g2_d[:, :])
        b2_t = consts.tile([P, EO], F32)
        nc.gpsimd.dma_start(b2_t[:], b2_d[:, :])
        ones_col = consts.tile([P, 1], BF16)
        nc.vector.memset(ones_col[:], 1.0 / E)
        eps_t = consts.tile([1, 1], F32)
        nc.vector.memset(eps_t[:], LN_EPS / (SA * SA))
        eps2_t = consts.tile([1, 1], F32)
        nc.vector.memset(eps2_t[:], LN_EPS)
        negmu_t = consts.tile([P, 1], F32)
        nc.vector.memset(negmu_t[:], -MU * SA)
        sa_t = consts.tile([P, 1], F32)
        nc.vector.memset(sa_t[:], SA)

        # ---- x digit tiles: groups of 4 rows (Ah,Ah',Al,Al') per pair ----
        xds = []
        for t in range(NT):
            xd = d32p.tile([P, 32, TS[t]], F8, tag=f"d32_{t}", name=f"xd{t}")
            xds.append(xd)
        dmaq = (nc.scalar, nc.gpsimd)

        def load_xd(t):
            if t == 0:
                # one DMA: block 0 needs every row before its PSUM group
                # completes, and a single transfer saves 7 queue overheads
                nc.scalar.dma_start(xds[t][:], xd_d[:, :, tsl(t)])
                return
            for g in range(8):
                dmaq[g % 2].dma_start(xds[t][:, 4 * g:4 * g + 4, :],
                                      xd_d[:, 4 * g:4 * g + 4, tsl(t)])

        load_xd(0)

        # warm MMs: keep PE busy while the first DMAs land (p-state ramp)
        warm_rhs = consts.tile([P, 512], BF16)
        nc.vector.memset(warm_rhs[:], 1.0)
        warm_ps = psst.tile([1, 512], F32, tag="pss")
        for _ in range(14):
            nc.tensor.matmul(warm_ps[:], lhsT=ones_col[:],
                             rhs=warm_rhs[:], start=True, stop=True)

        def hilo_group(ps, wt, rows, T):
            """24 DR passes: (Ah+Al)@Wh + Ah@Wl for 8 chunk pairs."""
            for g in range(8):
                hi = rows[:, 4 * g:4 * g + 2, :]
                lo = rows[:, 4 * g + 2:4 * g + 4, :]
                nc.tensor.matmul(ps[:, :T], lhsT=wt[:, 2 * g, :, :], rhs=hi,
                                 start=(g == 0), stop=False, perf_mode=DRM)
                nc.tensor.matmul(ps[:, :T], lhsT=wt[:, 2 * g, :, :], rhs=lo,
                                 start=False, stop=False, perf_mode=DRM)
                nc.tensor.matmul(ps[:, :T], lhsT=wt[:, 2 * g + 1, :, :], rhs=hi,
                                 start=False, stop=(g == 7), perf_mode=DRM)

        def ln_finalize(t, ps_sum, ps_sq, bc_out=None, coff=0,
                        for_ln2=False, defer_bcast=False):
            """bc [P,4,T] bf16: rows (m*rstd*SA, m*rstd*SA, rstd*SA, rstd*SA)
            so chunk-PAIR ops can slice bc[:,0:2] / bc[:,2:4] directly.
            ps_sum/ps_sq arrive pre-divided by E (ones = 1/E)."""
            T = TS[t]
            st = stp.tile([1, 4, 512], BF16, tag="st")
            nc.vector.tensor_scalar_mul(st[:, 1, :T], ps_sum[:, :T], 1.0)
            nc.vector.tensor_mul(out=st[:, 3, :T], in0=st[:, 1, :T],
                                 in1=st[:, 1, :T])
            nc.vector.tensor_tensor(st[:, 2, :T], ps_sq[:, :T], st[:, 3, :T],
                                    ALU.subtract)
            if for_ln2:
                nc.scalar.activation(st[:, 3, :T], st[:, 2, :T], AF.Sqrt,
                                     bias=eps2_t[:], scale=1.0)
            else:
                nc.scalar.activation(st[:, 3, :T], st[:, 2, :T], AF.Sqrt,
                                     bias=eps_t[:], scale=1.0 / (SA * SA))
            with nc.allow_low_precision(reason="bf16 rstd: ~0.2% scale "
                                        "error, well inside the fp8 budget"):
                nc.vector.reciprocal(out=st[:, 2, :T], in_=st[:, 3, :T])
            nc.vector.tensor_scalar_mul(st[:, 3, :T], st[:, 2, :T], 1.0)
            nc.vector.tensor_mul(out=st[:, 0, :T], in0=st[:, 1, :T],
                                 in1=st[:, 2, :T])
            nc.vector.tensor_scalar_mul(st[:, 1, :T], st[:, 0, :T], 1.0)
            if defer_bcast:
                return st
            bc = bc_out
            if bc is None:
                bc = bcp.tile([P, 4, 512], BF16, tag="bc")
            # rstd rows first: the chunk-pair muls only need bc[:,2:4]
            nc.gpsimd.partition_broadcast(bc[:, 2:4, coff:coff + T],
                                          st[:, 2:4, :T])
            nc.gpsimd.partition_broadcast(bc[:, 0:2, coff:coff + T],
                                          st[:, 0:2, :T])
            return bc

        def stats_chain(run, t, val_f32, m):
            """bf16 running sums of values (DVE) and squares (ACT + DVE)."""
            T = TS[t]
            if m == 0:
                pr = accp.tile([P, TS[t]], BF16, tag=f"pr{t}")
                nc.vector.tensor_scalar_mul(pr[:, :T], val_f32, 1.0)
                pq = accp.tile([P, TS[t]], BF16, tag=f"pq{t}")
                nc.scalar.activation(pq[:, :T], val_f32, AF.Square)
                run["pr"], run["pq"] = pr, pq
            else:
                sq = sqp.tile([P, 512], BF16, tag="sq", bufs=3)
                nc.scalar.activation(sq[:, :T], val_f32, AF.Square)
                nc.vector.tensor_tensor(run["pr"][:, :T], run["pr"][:, :T],
                                        val_f32, ALU.add)
                nc.vector.tensor_tensor(run["pq"][:, :T], run["pq"][:, :T],
                                        sq[:, :T], ALU.add)

        def stats_mms(run, t):
            T = TS[t]
            ps_sum = psst.tile([1, 512], F32, tag="pss")
            ps_sq = psst.tile([1, 512], F32, tag="psq")
            nc.tensor.matmul(ps_sum[:, :T], lhsT=ones_col[:],
                             rhs=run["pr"][:, :T], start=True, stop=True)
            nc.tensor.matmul(ps_sq[:, :T], lhsT=ones_col[:],
                             rhs=run["pq"][:, :T], start=True, stop=True)
            return ps_sum, ps_sq

        # ---------------- LN1 finalize + h digits (per tile) -------------
        # h digit rows mirror the x layout: chunk c -> hi row 4*(c//2)+(c%2),
        # lo row = hi row + 2. LN1's gamma/beta are folded into wfc/bfc on the
        # host, so digits quantize the bare normalized SA*h (bf16 chain, 2x).
        hds = [None] * NT
        pending = []   # deferred digit-pair closures, drained 1-3 per block

        def drain(n):
            for _ in range(min(n, len(pending))):
                pending.pop(0)()

        def emit_ln1_digits(t):
            sA, qA = stats_mms(runA[t], t)
            bc = ln_finalize(t, sA, qA)
            T = TS[t]
            hd = d32p.tile([P, 32, T], F8, tag=f"d32_{t}", name=f"hd{t}")
            hds[t] = hd

            def digit_pair(g, t=t, bc=bc, hd=hd, T=T):
                tm = sqp.tile([P, 2, 512], BF16, tag="dig", bufs=3)
                nc.vector.tensor_mul(out=tm[:, :, :T],
                                     in0=r1s[t][:, 2 * g:2 * g + 2, :],
                                     in1=bc[:, 2:4, :T])
                t2 = sqp.tile([P, 2, 512], BF16, tag="dig", bufs=3)
                nc.vector.tensor_tensor(t2[:, :, :T], tm[:, :, :T],
                                        bc[:, 0:2, :T], ALU.subtract)
                nc.scalar.activation(hd[:, 4 * g:4 * g + 2, :], t2[:, :, :T],
                                     AF.Identity)
                nc.vector.tensor_tensor(hd[:, 4 * g + 2:4 * g + 4, :],
                                        t2[:, :, :T],
                                        hd[:, 4 * g:4 * g + 2, :],
                                        ALU.subtract)
            for g in range(EO // 2):
                pending.append(lambda g=g: digit_pair(g))

        # ---------------- phase A: kv matmul + residual ----------------
        # tile offsets (0,1,2): tile t processes block mb - t, so t0 finishes
        # early and its LN1/digit chain overlaps the other tiles' tail blocks.
        r1s, runA = [], [dict() for _ in range(NT)]
        for t in range(NT):
            r1 = bigp.tile([P, EO, TS[t]], BF16, tag=f"big_{t}", name=f"r1{t}")
            r1s.append(r1)
        def a_block(t, m, wt):
            T = TS[t]
            ps = psmm.tile([P, 512], F32, tag="ps")
            hilo_group(ps, wt, xds[t], T)
            xc = xbp.tile([P, 512], BF16, tag="xb")
            (nc.sync if t == 0 else nc.gpsimd).dma_start(
                xc[:, :T], xb_d[m * P:(m + 1) * P, tsl(t)])
            t1 = f32t.tile([P, 512], F32, tag="f32")
            nc.scalar.activation(t1[:, :T], ps[:, :T], AF.Identity,
                                 bias=bkv_t[:, m:m + 1], scale=1.0 / PS)
            nc.vector.tensor_add(out=r1s[t][:, m, :], in0=t1[:, :T],
                                 in1=xc[:, :T])
            stats_chain(runA[t], t, r1s[t][:, m, :], m)

        # pass 1: tile 0 alone; its LN1 + digit chain then overlaps pass 2.
        wts_a = {}
        for m in range(EO):
            wt = wp.tile([P, EO, 2, P], F8, tag="w16")
            nc.sync.dma_start(wt[:], wkv_d[:, m])
            a_block(0, m, wt)
            if m == 6:
                load_xd(1)
            if m == 10:
                load_xd(2)
            if m >= 12:
                pwt = wp.tile([P, EO, 2, P], F8, tag="w16", name="pwt")
                wts_a[m - 12] = pwt
                nc.sync.dma_start(pwt[:], wkv_d[:, m - 12])
        emit_ln1_digits(0)
        # pass 2: tiles 1,2 lockstep (t2 lagging one block); kv weights are
        # cheap enough to stream a second time (first two prefetched above).
        for mb in range(EO + 1):
            if 4 <= mb < EO:
                wt = wp.tile([P, EO, 2, P], F8, tag="w16")
                wts_a[mb] = wt
                nc.sync.dma_start(wt[:], wkv_d[:, mb])
            for t, off in ((1, 0), (2, 1)):
                m = mb - off
                if not (0 <= m < EO):
                    continue
                a_block(t, m, wts_a[m])
                if m == EO - 1:
                    emit_ln1_digits(t)
            if mb % 2 == 1:
                drain(1)


        # ---------------- phase B1: fc matmul + gelu + u digits ----------
        uds = []
        for t in range(NT):
            ud = bigp.tile([P, NUR, TS[t]], F8, tag=f"big_{t}", name=f"ud{t}")
            uds.append(ud)
        FSKIP = (0, 8, 10)

        def fc_block(t, ma, wt):
                T = TS[t]
                ps = psmm.tile([P, 512], F32, tag="ps")
                hilo_group(ps, wt, hds[t], T)
                if ma < J2:
                    uf = f32t.tile([P, 512], F32, tag="f32")
                    nc.scalar.activation(uf[:, :T], ps[:, :T], AF.Gelu,
                                         bias=bfc_t[:, ma:ma + 1],
                                         scale=1.0 / PS)
                    nc.scalar.activation(uds[t][:, ma, :], uf[:, :T],
                                         AF.Identity, bias=negmu_t[:],
                                         scale=SA)
                    t3 = f32t.tile([P, 512], F32, tag="f32")
                    nc.vector.tensor_scalar(t3[:, :T], uf[:, :T], sa_t[:],
                                            negmu_t[:], ALU.mult, ALU.add)
                    nc.vector.tensor_tensor(uds[t][:, J2 + ma, :], t3[:, :T],
                                            uds[t][:, ma, :], ALU.subtract)
                else:
                    nc.scalar.activation(uds[t][:, 2 * J2 + (ma - J2), :],
                                         ps[:, :T], AF.Gelu,
                                         bias=bfc_t[:, ma:ma + 1],
                                         scale=1.0 / PS)

        for mb in range(FO):
            wt = wp.tile([P, EO, 2, P], F8, tag="w16")
            nc.sync.dma_start(wt[:], wfc_d[:, mb])
            for t in range(NT):
                if mb >= FSKIP[t]:
                    fc_block(t, mb, wt)
            drain(2)
        # catch-up: the first blocks t1/t2 skipped, with re-streamed weights
        for cb in range(max(FSKIP)):
            wt = wp.tile([P, EO, 2, P], F8, tag="w16")
            nc.sync.dma_start(wt[:], wfc_d[:, cb])
            for t in (1, 2):
                if cb < FSKIP[t]:
                    fc_block(t, cb, wt)

        # -------- phase B2 + C: mproj + LN2 + output, two stages --------
        # stage 2 (t1,t2) shares one 512-column v2f tile and one bc tile so
        # the exposed tail normalizes and stores both tiles full-width.
        v2fs = [None] * NT
        OFFV = (0, 0, TS[1])
        bc2s = [None] * NT
        outq = (nc.gpsimd, nc.sync)

        def phase_c_begin(t, runB):
            sB, qB = stats_mms(runB, t)
            bc2s[t] = ln_finalize(t, sB, qB, for_ln2=True)

        def phase_c_pair(t, g, tailq=False, wide=False):
            """normalize chunks 2g, 2g+1 of tile t in-place into the (dead)
            v2f rows; with wide=True the op covers the merged t1|t2 columns
            and stores go one-per-chunk across both tiles."""
            T = 512 if wide else TS[t]
            co = 0 if wide else OFFV[t]
            bc = bc2s[t]
            q = (nc.sync, nc.scalar, nc.gpsimd) if tailq else outq
            vsl = v2fs[t][:, 2 * g:2 * g + 2, co:co + T]
            tm = sqp.tile([P, 2, 512], BF16, tag="dig", bufs=3)
            nc.vector.tensor_mul(out=tm[:, :, :T], in0=vsl,
                                 in1=bc[:, 2:4, co:co + T])
            if LN2_TRIVIAL:
                nc.vector.tensor_tensor(vsl, tm[:, :, :T],
                                        bc[:, 0:2, co:co + T], ALU.subtract)
            else:
                for i in (0, 1):
                    m = 2 * g + i
                    t2 = sqp.tile([P, 512], BF16, tag="dg1")
                    nc.vector.tensor_tensor(t2[:, :T], tm[:, i, :T],
                                            bc[:, i, co:co + T], ALU.subtract)
                    nc.scalar.activation(v2fs[t][:, m, co:co + T], t2[:, :T],
                                         AF.Identity, bias=b2_t[:, m:m + 1],
                                         scale=g2_t[:, m:m + 1])
            cs = slice(TOFF[1], TOK) if wide else tsl(t)
            for i in (0, 1):
                m = 2 * g + i
                q[m % len(q)].dma_start(out_d[m * P:(m + 1) * P, cs],
                                        v2fs[t][:, m, co:co + T])

        def mp_pass_group(ps, wta, wtb, t, T):
            def cell(j):
                return wta[:, j, :, :] if j < EO else wtb[:, j - EO, :, :]

            for j in range(J2 // 2):
                nc.tensor.matmul(ps[:, :T], lhsT=cell(j),
                                 rhs=uds[t][:, 2 * j:2 * j + 2, :],
                                 start=(j == 0), stop=False, perf_mode=DRM)
                nc.tensor.matmul(ps[:, :T], lhsT=cell(j),
                                 rhs=uds[t][:, J2 + 2 * j:J2 + 2 * j + 2, :],
                                 start=False, stop=False, perf_mode=DRM)
            for p in range(JP // 2):
                nc.tensor.matmul(ps[:, :T], lhsT=cell(J2 // 2 + p),
                                 rhs=uds[t][:, 2 * J2 + 2 * p:
                                            2 * J2 + 2 * p + 2, :],
                                 start=False, stop=(p == JP // 2 - 1),
                                 perf_mode=DRM)

        prev_tiles = []
        prev_runB = {}
        for stage in ((0,), (1, 2)):
            runB = {t: dict() for t in stage}
            if stage == (0,):
                v2fs[0] = d32p.tile([P, EO, TS[0]], BF16, tag="d32_0",
                                    name="v2f0")
            else:
                v2f12 = bigp.tile([P, EO, 512], BF16, tag="big_0",
                                  name="v2f12")
                v2fs[1] = v2f12
                v2fs[2] = v2f12
            for mo in range(EO):
                wta = wp.tile([P, EO, 2, P], F8, tag="w16", name="wta")
                nc.sync.dma_start(wta[:], wmp_d[:, mo, :EO])
                wtb = wp.tile([P, EO, 2, P], F8, tag="w16", name="wtb")
                nc.sync.dma_start(wtb[:], wmp_d[:, mo, EO:])
                for t in stage:
                    T = TS[t]
                    co = OFFV[t]
                    ps = psmm.tile([P, 512], F32, tag="ps")
                    mp_pass_group(ps, wta, wtb, t, T)
                    xc = xbp.tile([P, 512], BF16, tag="xb")
                    nc.gpsimd.dma_start(xc[:, :T],
                                        xb_d[mo * P:(mo + 1) * P, tsl(t)])
                    tv = f32t.tile([P, 512], F32, tag="f32")
                    nc.scalar.activation(tv[:, :T], ps[:, :T], AF.Identity,
                                         bias=bmp_t[:, mo:mo + 1],
                                         scale=1.0 / PS)
                    vrow = v2fs[t][:, mo, co:co + T]
                    nc.vector.tensor_add(out=vrow, in0=tv[:, :T],
                                         in1=xc[:, :T])
                    stats_chain(runB[t], t, vrow, mo)
                for tp in prev_tiles:
                    if mo == 0:
                        phase_c_begin(tp, prev_runB[tp])
                    if mo % 2 == 0:
                        phase_c_pair(tp, mo // 2)

            prev_tiles = list(stage)
            prev_runB = runB
        # tail: LN2 + normalize + store for the merged t1|t2 tile. Both
        # tiles' rstd rows broadcast before the mean rows so the first wide
        # mul is gated on as little as possible; pair ops run in place.
        bc12 = bcp.tile([P, 4, 512], BF16, tag="bc")
        sts = {}
        for tp in prev_tiles:
            sB, qB = stats_mms(prev_runB[tp], tp)
            sts[tp] = ln_finalize(tp, sB, qB, for_ln2=True, defer_bcast=True)
            bc2s[tp] = bc12
        for rows in ((2, 4), (0, 2)):
            for tp in prev_tiles:
                Tt = TS[tp]
                nc.gpsimd.partition_broadcast(
                    bc12[:, rows[0]:rows[1], OFFV[tp]:OFFV[tp] + Tt],
                    sts[tp][:, rows[0]:rows[1], :Tt])
        tq = (nc.sync, nc.scalar, nc.gpsimd)
        if LN2_TRIVIAL:
            for g in range(EO // 2):
                vsl = v2f12[:, 2 * g:2 * g + 2, :]
                nc.vector.tensor_mul(out=vsl, in0=vsl, in1=bc12[:, 2:4, :])
                nc.vector.tensor_tensor(vsl, vsl, bc12[:, 0:2, :],
                                        ALU.subtract)
                for i in (0, 1):
                    m = 2 * g + i
                    tq[m % 3].dma_start(out_d[m * P:(m + 1) * P,
                                              slice(TOFF[1], TOK)],
                                        v2f12[:, m, :])
        else:
            for g in range(EO // 2):
                phase_c_pair(1, g, tailq=True, wide=True)

    nc.compile()
    return nc


def _get_nc(ln2_trivial=True):
    if ln2_trivial not in _CACHED_NC:
        _CACHED_NC[ln2_trivial] = _build(ln2_trivial)
    return _CACHED_NC[ln2_trivial]


def _q(x):
    return x.astype(E4NP)


def _dec(x):
    return x.astype(np.float32)


def _prep_inputs(x, w_kv, b_kv, w_fc, b_fc, w_mproj, b_mproj,
                 ln1_g, ln1_b, ln2_g, ln2_b):
    f32 = np.float32
    x_flat = np.ascontiguousarray(np.asarray(x, f32).reshape(B * S, E))

    def pack_hilo(W, KO, NO):
        """-> [P, NO, KO, 2, P] fp8; pair 2j=(Wh_2j,Wh_2j+1), 2j+1=lo pair."""
        Ws = (np.asarray(W, f32) * SW).reshape(KO, P, NO, P)
        Wh = _q(Ws)
        Wl = _q(Ws - _dec(Wh))
        Whp = Wh.reshape(KO // 2, 2, P, NO, P).transpose(3, 2, 0, 1, 4)
        Wlp = Wl.reshape(KO // 2, 2, P, NO, P).transpose(3, 2, 0, 1, 4)
        arr = np.stack([Whp, Wlp], axis=3)          # [NO,P,KO/2,2,2,P]
        arr = arr.reshape(NO, P, KO, 2, P).transpose(1, 0, 2, 3, 4)
        return np.ascontiguousarray(arr)            # [P,NO,KO,2,P]

    wkv_t = pack_hilo(w_kv, EO, EO)
    # fold LN1 affine into fc: z = (g1*hhat + b1) @ wfc + bfc
    #   = hhat @ (g1[:,None]*wfc) + (bfc + b1 @ wfc); digits quantize SA*hhat.
    g1 = np.asarray(ln1_g, f32)
    b1 = np.asarray(ln1_b, f32)
    wfc_f = np.asarray(w_fc, f32) * g1[:, None]
    bfc_f = np.asarray(b_fc, f32) + b1 @ np.asarray(w_fc, f32)
    wfc_t = pack_hilo(wfc_f, EO, FO)

    Wmp = np.asarray(w_mproj, f32).reshape(FO, P, EO, P)
    wc = Wmp[:J2] * SW
    w8c = _q(wc)
    wpl = Wmp[J2:] * (SA * SW)
    w8p = _q(wpl)
    pairs_c = _dec(w8c).reshape(J2 // 2, 2, P, EO, P).transpose(3, 2, 0, 1, 4)
    pairs_p = _dec(w8p).reshape(JP // 2, 2, P, EO, P).transpose(3, 2, 0, 1, 4)
    wmp_t = np.concatenate([_q(pairs_c), _q(pairs_p)], axis=2)
    wmp_t = np.ascontiguousarray(wmp_t.transpose(1, 0, 2, 3, 4))

    # host bias corrections (data-free)
    err_c = (wc - _dec(w8c)).sum((0, 1)) / SW            # [EO, P]
    err_p = (wpl - _dec(w8p)).sum((0, 1)) / (SA * SW)
    dec_c = _dec(w8c).sum((0, 1)) / SW
    bcorr = MU * (err_c + err_p) + MU * dec_c            # [EO, P]
    bmp_c = np.asarray(b_mproj, f32).reshape(EO, P) + bcorr

    def p2d(v):
        v = np.asarray(v, f32)
        return np.ascontiguousarray(v.reshape(-1, P).T)

    shared = {
        "wkv": wkv_t, "wfc": wfc_t, "wmp": wmp_t,
        "bkv": p2d(b_kv), "bfc": p2d(bfc_f),
        "bmp": np.ascontiguousarray(bmp_c.T),
        "g2": p2d(ln2_g), "b2": p2d(ln2_b),
    }
    in_maps = []
    for c in range(NCORES):
        xT = np.ascontiguousarray(x_flat[c * TOK:(c + 1) * TOK].T)  # [E, TOK]
        xs = xT * np.float32(SA)
        xh = _q(xs)
        xl = _q(xs - _dec(xh))
        xh = xh.reshape(EO, P, TOK)
        xl = xl.reshape(EO, P, TOK)
        # group-of-4 rows: [Ah_2g, Ah_2g+1, Al_2g, Al_2g+1]
        xd = np.empty((32, P, TOK), E4NP)
        for g in range(8):
            xd[4 * g] = xh[2 * g]
            xd[4 * g + 1] = xh[2 * g + 1]
            xd[4 * g + 2] = xl[2 * g]
            xd[4 * g + 3] = xl[2 * g + 1]
        in_maps.append({
            "xb": xT.astype(ml_dtypes.bfloat16),
            "xd": np.ascontiguousarray(xd.transpose(1, 0, 2)),
            **shared})
    return in_maps


def _run(inputs, trace=False):
    ln2_trivial = bool(np.all(np.asarray(inputs["ln2_g"]) == 1.0)
                       and np.all(np.asarray(inputs["ln2_b"]) == 0.0))
    nc = _get_nc(ln2_trivial)
    in_maps = _prep_inputs(
        inputs["x"], inputs["w_kv"], inputs["b_kv"], inputs["w_fc"],
        inputs["b_fc"], inputs["w_mproj"], inputs["b_mproj"],
        inputs["ln1_g"], inputs["ln1_b"], inputs["ln2_g"], inputs["ln2_b"])
    res = run_bass_kernel_spmd(nc, in_maps, core_ids=list(range(NCORES)),
                               trace=trace)
    outs = [np.asarray(res.results[c]["out"]).astype(np.float32).T
            for c in range(NCORES)]
    full = np.concatenate(outs, axis=0).reshape(B, S, E)
    return full, res


def kernel(**inputs) -> np.ndarray:
    out, _ = _run(inputs, trace=False)
    return out
